# revision 1
# baseline (speedup 1.0000x reference)
"""DeeperHNN hypergraph message passing kernel for 8 Trainium2 NeuronCores.

Strategy (sharding_hint): nodes (and incidence entries, partitioned by vertex)
are sharded across 8 cores; hyperedge aggregates are computed as per-core
partials and AllReduced (replicated) since E << N; weight matrices replicated.

Per conv layer, per core:
  T = h @ thetaW[i] + thetaB[i]                     (row-major, DRAM)
  Phase A: partial_Ye[e] = sum_{local v in e} de_inv[e] * T[v]
           = gather T rows by vidx (dma_gather) -> one-hot segment matmul on PE
           -> contiguous 128-edge block writes
  AllReduce(partial_Ye) -> Ye (replicated)
  Phase B: conv[v] = relu(dv_inv[v] * sum_{e: v in e} Ye[e])
           = gather Ye rows by eidx -> one-hot segment matmul -> 128-node blocks
  h' = h + conv (residual, layers 1-3); tail: t = relu(LN(h')), hT = t^T for the
  next layer's matmul.

All segment structure (slot counts, token index tables, one-hot row ids,
de_inv weights) is precomputed on host from vidx/eidx and shipped as int16/f32
metadata tables resident in SBUF.
"""

import math

import numpy as np

import concourse.bacc as bacc
import concourse.bass as bass
import concourse.mybir as mybir
import concourse.tile as tile
from concourse.bass_utils import run_bass_kernel_spmd
from concourse.masks import make_identity

import ml_dtypes

P = 128
BF16_NP = ml_dtypes.bfloat16
USE_BF16 = True
F32 = mybir.dt.float32
BF16 = mybir.dt.bfloat16
I16 = mybir.dt.int16
I32 = mybir.dt.int32
AF = mybir.ActivationFunctionType
ALU = mybir.AluOpType


def _cdiv(a, b):
    return (a + b - 1) // b


# ----------------------------------------------------------------------------
# Host-side preprocessing: build per-core token tables from vidx/eidx.
# ----------------------------------------------------------------------------
class Prep:
    pass


def host_prep(vidx, eidx, N, E, C, gmax_a=24, gmax_b=20):
    """Build the static segment/gather structure shared by the SPMD program.

    Tokens for phase A (v->e): per core, entries sorted by eidx, grouped into
    157 blocks of 128 edges; each block padded to a whole number of 128-token
    slots (count = max over cores, so the program is identical on all cores).
    Phase B (e->v) is the same with the roles of (node block, eidx) swapped.
    """
    p = Prep()
    NP = N // C
    NBE = _cdiv(E, P)
    NBV = _cdiv(NP, P)
    NPAD = NBV * P
    EPAD = NBE * P
    p.N, p.E, p.C, p.NP, p.NBE, p.NBV, p.NPAD, p.EPAD = N, E, C, NP, NBE, NBV, NPAD, EPAD

    vidx = np.asarray(vidx).astype(np.int64)
    eidx = np.asarray(eidx).astype(np.int64)
    de = np.bincount(eidx, minlength=E).astype(np.float64)
    dv = np.bincount(vidx, minlength=N).astype(np.float64)
    de_inv = (1.0 / np.maximum(de, 1.0)).astype(np.float32)
    dv_inv = (1.0 / np.maximum(dv, 1.0)).astype(np.float32)
    core = vidx // NP

    # ---- phase A ----
    A_ev, A_lv = [], []
    cntA = np.zeros((C, NBE), np.int64)
    for c in range(C):
        m = core == c
        ev = eidx[m]
        lv = vidx[m] - c * NP
        o = np.argsort(ev, kind="stable")
        ev, lv = ev[o], lv[o]
        cntA[c] = np.bincount(ev // P, minlength=NBE)
        A_ev.append(ev)
        A_lv.append(lv)
    slotsA = np.maximum(1, _cdiv(cntA.max(0), P)).astype(np.int64)
    SA = int(slotsA.sum())
    TA = SA * P
    offA = np.zeros(NBE + 1, np.int64)
    np.cumsum(slotsA * P, out=offA[1:])

    idxA = np.full((C, TA), NP, np.int16)  # dummy -> zero row of T
    rA = np.zeros((C, TA), np.float32)
    for c in range(C):
        ev, lv = A_ev[c], A_lv[c]
        blk = ev // P
        starts = np.searchsorted(ev, np.arange(NBE) * P)
        tok = offA[blk] + (np.arange(len(ev)) - starts[blk])
        idxA[c, tok] = lv
        rA[c, tok] = ev - blk * P

    # ---- phase B ----
    B_ee, B_lv = [], []
    cntB = np.zeros((C, NBV), np.int64)
    for c in range(C):
        m = core == c
        lv = vidx[m] - c * NP
        ee = eidx[m]
        o = np.argsort(lv, kind="stable")
        lv, ee = lv[o], ee[o]
        cntB[c] = np.bincount(lv // P, minlength=NBV)
        B_ee.append(ee)
        B_lv.append(lv)
    slotsB = np.maximum(1, _cdiv(cntB.max(0), P)).astype(np.int64)
    SB = int(slotsB.sum())
    TB = SB * P
    offB = np.zeros(NBV + 1, np.int64)
    np.cumsum(slotsB * P, out=offB[1:])

    idxB = np.full((C, TB), E, np.int16)  # dummy -> zeroed row E of Ye
    rB = np.zeros((C, TB), np.float32)
    for c in range(C):
        ee, lv = B_ee[c], B_lv[c]
        blk = lv // P
        starts = np.searchsorted(lv, np.arange(NBV) * P)
        tok = offB[blk] + (np.arange(len(lv)) - starts[blk])
        idxB[c, tok] = ee
        rB[c, tok] = lv - blk * P

    # per-node dv_inv columns [C, 128, NBV]
    dvc = np.zeros((C, P, NBV), np.float32)
    for c in range(C):
        ids = c * NP + np.arange(NPAD)
        vals = np.where(ids < (c + 1) * NP, dv_inv[np.minimum(ids, N - 1)], 0.0)
        dvc[c] = vals.reshape(NBV, P).T

    # wrapped layouts for the device
    p.slotsA, p.slotsB, p.SA, p.SB, p.TA, p.TB = slotsA, slotsB, SA, SB, TA, TB
    p.offA, p.offB = offA, offB
    p.idxA_w = np.ascontiguousarray(np.tile(idxA.reshape(C, TA // 16, 16).transpose(0, 2, 1), (1, 8, 1)))
    rdt = BF16_NP if USE_BF16 else np.float32
    p.rA_m = np.ascontiguousarray(rA.reshape(C, SA, P).transpose(0, 2, 1)).astype(rdt)
    p.idxB_w = np.ascontiguousarray(np.tile(idxB.reshape(C, TB // 16, 16).transpose(0, 2, 1), (1, 8, 1)))
    p.rB_m = np.ascontiguousarray(rB.reshape(C, SB, P).transpose(0, 2, 1)).astype(rdt)
    # de_inv per edge-block column [128, NBE] (same on all cores)
    dec = np.zeros(EPAD, np.float32)
    dec[:E] = de_inv
    p.dec = dec.reshape(NBE, P).T.copy()
    p.dvc = dvc
    p.MAXSLOT = int(max(slotsA.max(), slotsB.max()))

    # gather groups: consecutive blocks, total slots <= gmax
    def make_groups(slots, gmax):
        groups = []  # (block0, nblocks, slot0, gslots)
        b = 0
        nb = len(slots)
        while b < nb:
            s0 = int(slots[:b].sum())
            g = 0
            n = 0
            while b + n < nb and g + slots[b + n] <= gmax:
                g += int(slots[b + n])
                n += 1
            assert n > 0, "single block exceeds gmax"
            groups.append((b, n, s0, g))
            b += n
        return groups

    p.gmax_a, p.gmax_b = gmax_a, gmax_b
    p.groupsA = make_groups(slotsA, gmax_a)
    p.groupsB = make_groups(slotsB, gmax_b)
    return p


# ----------------------------------------------------------------------------
# Device program
# ----------------------------------------------------------------------------
def build_program(p, IN_DIM, H, OUT, L, enable_asserts=False, stage=99):
    C, NP, NBE, NBV, NPAD, EPAD = p.C, p.NP, p.NBE, p.NBV, p.NPAD, p.EPAD
    KI = IN_DIM // P  # input-dim K tiles (3)
    KH = H // P  # hidden K tiles (2)
    assert IN_DIM % P == 0 and H % P == 0

    nc = bacc.Bacc(
        "TRN2",
        target_bir_lowering=False,
        debug=False,
        enable_asserts=enable_asserts,
        num_devices=C,
        num_swdge_queues=4,
    )

    # ---- I/O ----
    xT_d = nc.dram_tensor("xT", [IN_DIM, NPAD], F32, kind="ExternalInput")
    encW_d = nc.dram_tensor("encW", [IN_DIM, H], F32, kind="ExternalInput")
    encB_d = nc.dram_tensor("encB", [H], F32, kind="ExternalInput")
    thW_d = nc.dram_tensor("thW", [L, H, H], F32, kind="ExternalInput")
    thB_d = nc.dram_tensor("thB", [L, H], F32, kind="ExternalInput")
    lnG_d = nc.dram_tensor("lnG", [L, H], F32, kind="ExternalInput")
    lnB_d = nc.dram_tensor("lnB", [L, H], F32, kind="ExternalInput")
    linW_d = nc.dram_tensor("linW", [H, OUT], F32, kind="ExternalInput")
    linB_d = nc.dram_tensor("linB", [OUT], F32, kind="ExternalInput")
    idxA_d = nc.dram_tensor("idxA", [P, p.TA // 16], I16, kind="ExternalInput")
    GDT = BF16 if USE_BF16 else F32
    rA_d = nc.dram_tensor("rA", [P, p.SA], GDT, kind="ExternalInput")
    idxB_d = nc.dram_tensor("idxB", [P, p.TB // 16], I16, kind="ExternalInput")
    rB_d = nc.dram_tensor("rB", [P, p.SB], GDT, kind="ExternalInput")
    dv_d = nc.dram_tensor("dvc", [P, NBV], F32, kind="ExternalInput")
    dec_d = nc.dram_tensor("dec", [P, NBE], F32, kind="ExternalInput")
    out_d = nc.dram_tensor("out", [NP, OUT], F32, kind="ExternalOutput")

    # ---- internals ----
    TEXT = P if NP == NPAD else 0  # room for the dummy row when NP % 128 == 0
    EEXT = P if p.E == EPAD else 0
    T_d = nc.dram_tensor("T_t", [NPAD + TEXT, H], GDT)  # row NP is the zero dummy
    YeP_d = nc.dram_tensor("YeP", [EPAD, H], GDT)  # rows E.. end up zero
    YeF_d = nc.dram_tensor(
        "YeF", [EPAD + EEXT, H], GDT,
        addr_space="Shared" if C > 4 else "Local",
    )
    h_d = nc.dram_tensor("h_t", [NPAD, H], F32)
    hT_d = nc.dram_tensor("hT", [H, NPAD], F32)

    last_rows = NP - (NBV - 1) * P  # valid rows in the final node block

    from contextlib import ExitStack
    with tile.TileContext(nc) as tc, ExitStack() as es:
        const = es.enter_context(tc.tile_pool(name="const", bufs=1))
        meta = es.enter_context(tc.tile_pool(name="meta", bufs=1))
        gpool = es.enter_context(tc.tile_pool(name="gpool", bufs=2))
        spool = es.enter_context(tc.tile_pool(name="spool", bufs=3))
        wrk = es.enter_context(tc.tile_pool(name="wrk", bufs=3))
        stat = es.enter_context(tc.tile_pool(name="stat", bufs=4))
        opool = es.enter_context(tc.tile_pool(name="opool", bufs=3))
        psA = es.enter_context(tc.tile_pool(name="psA", bufs=3, space="PSUM"))
        psT = es.enter_context(tc.tile_pool(name="psT", bufs=2, space="PSUM"))
        psE = es.enter_context(tc.tile_pool(name="psE", bufs=2, space="PSUM"))

        # ---- constants ----
        iota_i = const.tile([P, p.MAXSLOT, P], I32)
        nc.gpsimd.iota(iota_i[:, :, :], pattern=[[0, p.MAXSLOT], [1, P]], base=0,
                       channel_multiplier=0)
        iota_f = const.tile([P, p.MAXSLOT, P], GDT)
        nc.vector.tensor_copy(iota_f[:, :, :], iota_i[:, :, :])
        ident = const.tile([P, P], F32)
        make_identity(nc, ident[:, :])
        ones1 = const.tile([1, P], F32)
        nc.vector.memset(ones1[:, :], 1.0)
        epsc = const.tile([P, 1], F32)
        nc.vector.memset(epsc[:, :], 1e-5)
        zrow = const.tile([1, H], F32)
        nc.vector.memset(zrow[:, :], 0.0)

        # weights
        encW_t = []
        for k in range(KI):
            row = []
            for m in range(KH):
                t = const.tile([P, P], F32, tag=f"encW{k}{m}")
                nc.sync.dma_start(t[:, :], encW_d[k * P:(k + 1) * P, m * P:(m + 1) * P])
                row.append(t)
            encW_t.append(row)
        encB_c = []
        for m in range(KH):
            t = const.tile([P, 1], F32, tag=f"encB{m}")
            nc.sync.dma_start(t[:, :], encB_d[m * P:(m + 1) * P, None])
            encB_c.append(t)
        thW_t = []
        for i in range(L):
            row = []
            for k in range(KH):
                t = const.tile([P, H], F32, tag=f"thW{i}{k}")
                nc.sync.dma_start(t[:, :], thW_d[i, k * P:(k + 1) * P, :])
                row.append(t)
            thW_t.append(row)
        thB_t = []
        for i in range(L):
            t = const.tile([1, H], F32, tag=f"thB{i}")
            nc.sync.dma_start(t[:, :], thB_d[i:i + 1, :])
            thB_t.append(t)
        linW_t = []
        for k in range(KH):
            t = const.tile([P, OUT], F32, tag=f"linW{k}")
            nc.sync.dma_start(t[:, :], linW_d[k * P:(k + 1) * P, :])
            linW_t.append(t)
        linB_t = const.tile([1, OUT], F32)
        nc.sync.dma_start(linB_t[:, :], linB_d[None, :])
        lnG_t, lnB_t = [], []
        for i in range(L):
            g = const.tile([P, H], F32, tag=f"lnG{i}")
            b = const.tile([P, H], F32, tag=f"lnB{i}")
            nc.sync.dma_start(g[:, :], lnG_d[i:i + 1, :].partition_broadcast(P).squeeze(1))
            nc.sync.dma_start(b[:, :], lnB_d[i:i + 1, :].partition_broadcast(P).squeeze(1))
            lnG_t.append(g)
            lnB_t.append(b)

        # metadata
        idxA_t = meta.tile([P, p.TA // 16], I16)
        nc.sync.dma_start(idxA_t[:, :], idxA_d[:, :])
        rA_t = meta.tile([P, p.SA], GDT)
        nc.sync.dma_start(rA_t[:, :], rA_d[:, :])
        dec_t = meta.tile([P, NBE], F32)
        nc.sync.dma_start(dec_t[:, :], dec_d[:, :])
        idxB_t = meta.tile([P, p.TB // 16], I16)
        nc.sync.dma_start(idxB_t[:, :], idxB_d[:, :])
        rB_t = meta.tile([P, p.SB], GDT)
        nc.sync.dma_start(rB_t[:, :], rB_d[:, :])
        dv_t = meta.tile([P, NBV], F32)
        nc.sync.dma_start(dv_t[:, :], dv_d[:, :])

        # zero the dummy/pad zones that gathers may read
        zblk = const.tile([P, H], GDT)
        nc.vector.memset(zblk[:, :], 0.0)
        r0 = NP
        while r0 < NPAD + TEXT:
            nr = min(P, NPAD + TEXT - r0)
            nc.sync.dma_start(T_d[r0:r0 + nr, :], zblk[:nr, :])
            r0 += nr
        if EEXT:
            nc.sync.dma_start(YeF_d[EPAD:EPAD + EEXT, :], zblk[:EEXT, :])

        # ------------------------------------------------------------------
        # Encoder: hT = (x @ encW + encB)^T, computed directly feature-major.
        # ------------------------------------------------------------------
        CW = 512
        for c0 in range(0, NPAD, CW):
            ncols = min(CW, NPAD - c0)
            xc = wrk.tile([P, KI, CW], F32, tag="xc")
            nc.sync.dma_start(
                xc[:, :, :ncols],
                xT_d.ap().rearrange("(k q) n -> q k n", q=P)[:, :, c0:c0 + ncols],
            )
            for m in range(KH):
                ps = psE.tile([P, CW], F32, tag="psE")
                for k in range(KI):
                    nc.tensor.matmul(ps[:, :ncols], lhsT=encW_t[k][m][:, :],
                                     rhs=xc[:, k, :ncols],
                                     start=(k == 0), stop=(k == KI - 1))
                ob = opool.tile([P, CW], F32, tag="encout")
                nc.scalar.activation(ob[:, :ncols], ps[:, :ncols], AF.Identity,
                                     bias=encB_c[m][:, :], scale=1.0)
                nc.scalar.dma_start(hT_d[m * P:(m + 1) * P, c0:c0 + ncols], ob[:, :ncols])

        hT_view = hT_d.ap().rearrange("(k q) n -> q k n", q=P)

        # ------------------------------------------------------------------
        # Conv layers
        # ------------------------------------------------------------------
        for li in range(L if stage >= 2 else 0):
            # ---- T = h @ thetaW[li] + thetaB[li] (row-major) ----
            for rb in range(NBV):
                rows = last_rows if rb == NBV - 1 else P
                hTl = wrk.tile([P, KH, P], F32, tag="hTl")
                nc.sync.dma_start(hTl[:, :, :], hT_view[:, :, rb * P:rb * P + P])
                ps = psA.tile([P, H], F32, tag="ps256")
                for k in range(KH):
                    nc.tensor.matmul(ps[:, :], lhsT=hTl[:, k, :], rhs=thW_t[li][k][:, :],
                                     start=(k == 0), stop=False)
                nc.tensor.matmul(ps[:, :], lhsT=ones1[:1, :], rhs=thB_t[li][:1, :],
                                 start=False, stop=True)
                Tb = opool.tile([P, H], GDT, tag="Tout")
                nc.scalar.activation(Tb[:, :], ps[:, :], AF.Copy)
                nc.sync.dma_start(T_d[rb * P:rb * P + rows, :], Tb[:rows, :])

            # ---- Phase A: partial Ye ----
            qn = 0
            for eb in (range(NBE) if stage >= 3 else []):
                sb = int(p.slotsA[eb])
                s0 = int(p.offA[eb]) // P
                G = gpool.tile([P, p.MAXSLOT, H], GDT, tag="G")
                g0 = 0
                while g0 < sb:
                    gs = min(8, sb - g0)
                    tok0 = (s0 + g0) * P
                    nc.gpsimd.dma_gather(
                        out_ap=G[:, g0:g0 + gs, :],
                        in_ap=T_d[:, :],
                        idxs_ap=idxA_t[:, tok0 // 16:(tok0 + gs * P) // 16],
                        num_idxs=gs * P,
                        num_idxs_reg=gs * P,
                        elem_size=H,
                        queue_num=qn,
                    )
                    qn = (qn + 1) % 4
                    g0 += gs
                S = spool.tile([P, p.MAXSLOT, P], GDT, tag="SA")
                rb_ap = rA_t[:, s0:s0 + sb].unsqueeze(2).broadcast_to([P, sb, P])
                nc.vector.tensor_tensor(S[:, :sb, :], iota_f[:, :sb, :], rb_ap,
                                        op=ALU.is_equal)
                ps = psA.tile([P, H], F32, tag="ps256")
                for s in range(sb):
                    nc.tensor.matmul(ps[:, :], lhsT=S[:, s, :], rhs=G[:, s, :],
                                     start=(s == 0), stop=(s == sb - 1))
                yeb = opool.tile([P, H], GDT, tag="yeg")
                nc.scalar.activation(yeb[:, :], ps[:, :], AF.Copy,
                                     scale=dec_t[:, eb:eb + 1])
                nc.sync.dma_start(YeP_d[eb * P:(eb + 1) * P, :], yeb[:, :])

            # ---- AllReduce hyperedge partials ----
            if stage < 4:
                continue
            nc.gpsimd.collective_compute(
                "AllReduce",
                ALU.add,
                replica_groups=[list(range(C))],
                ins=[YeP_d.ap()[:EPAD, :]],
                outs=[YeF_d.ap()[:EPAD, :]],
            )

            # ---- Phase B: conv output + residual + LN tail ----
            lnxt = li + 1 if li + 1 < L else 0
            for vb in (range(NBV) if stage >= 5 else []):
                rows = last_rows if vb == NBV - 1 else P
                sb = int(p.slotsB[vb])
                s0 = int(p.offB[vb]) // P
                G = gpool.tile([P, p.MAXSLOT, H], GDT, tag="G")
                g0 = 0
                while g0 < sb:
                    gs = min(8, sb - g0)
                    tok0 = (s0 + g0) * P
                    nc.gpsimd.dma_gather(
                        out_ap=G[:, g0:g0 + gs, :],
                        in_ap=YeF_d[:, :],
                        idxs_ap=idxB_t[:, tok0 // 16:(tok0 + gs * P) // 16],
                        num_idxs=gs * P,
                        num_idxs_reg=gs * P,
                        elem_size=H,
                        queue_num=qn,
                    )
                    qn = (qn + 1) % 4
                    g0 += gs
                if True:
                    S = spool.tile([P, p.MAXSLOT, P], GDT, tag="SA")
                    rb_ap = rB_t[:, s0:s0 + sb].unsqueeze(2).broadcast_to([P, sb, P])
                    nc.vector.tensor_tensor(S[:, :sb, :], iota_f[:, :sb, :], rb_ap,
                                            op=ALU.is_equal)
                    ps = psA.tile([P, H], F32, tag="ps256")
                    for s in range(sb):
                        nc.tensor.matmul(ps[:, :], lhsT=S[:, s, :], rhs=G[:, s, :],
                                         start=(s == 0), stop=(s == sb - 1))
                    # relu(dv * x) (== dv * relu(x), dv >= 0)
                    hn = wrk.tile([P, H], F32, tag="hn")
                    nc.scalar.activation(hn[:, :], ps[:, :], AF.Relu,
                                         scale=dv_t[:, vb:vb + 1])
                    if li > 0:
                        hp = wrk.tile([P, H], F32, tag="hp")
                        nc.scalar.dma_start(hp[:rows, :], h_d[vb * P:vb * P + rows, :])
                        nc.vector.tensor_add(hn[:rows, :], hn[:rows, :], hp[:rows, :])
                    nc.scalar.dma_start(h_d[vb * P:vb * P + rows, :], hn[:rows, :])

                    # tail: t = relu(LN_lnxt(hn)); hT = t^T
                    if stage < 6:
                        continue
                    st6 = stat.tile([P, 6], F32, tag="st6")
                    nc.vector.bn_stats(st6[:, :], hn[:, :])
                    mv = stat.tile([P, 2], F32, tag="mv")
                    nc.vector.bn_aggr(mv[:, :], st6[:, :])
                    rstd = stat.tile([P, 1], F32, tag="rstd")
                    nc.scalar.activation(rstd[:, :], mv[:, 1:2], AF.Sqrt,
                                         bias=epsc[:, :], scale=1.0)
                    rinv = stat.tile([P, 1], F32, tag="rinv")
                    nc.vector.reciprocal(rinv[:, :], rstd[:, :])
                    tt = wrk.tile([P, H], F32, tag="tt")
                    nc.vector.tensor_scalar(tt[:, :], hn[:, :], mv[:, 0:1], rinv[:, :],
                                            op0=ALU.subtract, op1=ALU.mult)
                    nc.vector.tensor_tensor(tt[:, :], tt[:, :], lnG_t[lnxt][:, :],
                                            op=ALU.mult)
                    nc.vector.tensor_tensor(tt[:, :], tt[:, :], lnB_t[lnxt][:, :],
                                            op=ALU.add)
                    nc.vector.tensor_scalar_max(tt[:, :], tt[:, :], 0.0)
                    tT = opool.tile([P, KH, P], F32, tag="tT")
                    for m in range(KH):
                        pst = psT.tile([P, P], F32, tag="psT")
                        nc.tensor.transpose(pst[:, :], tt[:, m * P:(m + 1) * P], ident[:, :])
                        nc.scalar.activation(tT[:, m, :], pst[:, :], AF.Copy)
                    nc.scalar.dma_start(
                        hT_view[:, :, vb * P:vb * P + rows],
                        tT[:, :, :rows],
                    )

        # ------------------------------------------------------------------
        # Final: out = t @ linW + linB  (t == relu(LN_0(h)) already in hT)
        # ------------------------------------------------------------------
        for rb in range(NBV):
            rows = last_rows if rb == NBV - 1 else P
            hTl = wrk.tile([P, KH, P], F32, tag="hTl")
            nc.sync.dma_start(hTl[:, :, :], hT_view[:, :, rb * P:rb * P + P])
            ps = psT.tile([P, OUT], F32, tag="psT")
            for k in range(KH):
                nc.tensor.matmul(ps[:, :], lhsT=hTl[:, k, :], rhs=linW_t[k][:, :],
                                 start=(k == 0), stop=False)
            nc.tensor.matmul(ps[:, :], lhsT=ones1[:1, :], rhs=linB_t[:1, :],
                             start=False, stop=True)
            ob = opool.tile([P, OUT], F32, tag="finout")
            nc.scalar.activation(ob[:, :], ps[:, :], AF.Copy)
            nc.sync.dma_start(out_d[rb * P:rb * P + rows, :], ob[:rows, :])

    nc.compile()
    return nc


# ----------------------------------------------------------------------------
# Full pipeline: prep + build + run
# ----------------------------------------------------------------------------
def run_full(x, vidx, eidx, encW, encB, thetaW, thetaB, lnG, lnB, linW, linB,
             N, E, C, trace=False, nc_cache=None, stage=99, **runkw):
    IN_DIM = x.shape[1]
    H = encW.shape[1]
    OUT = linW.shape[1]
    L = thetaW.shape[0]

    p = host_prep(np.asarray(vidx), np.asarray(eidx), N, E, C)
    nc = nc_cache if nc_cache is not None else build_program(p, IN_DIM, H, OUT, L, stage=stage)

    x = np.asarray(x, np.float32)
    NP, NPAD = p.NP, p.NPAD
    in_maps = []
    for c in range(C):
        xs = x[c * NP:(c + 1) * NP]
        xT = np.zeros((IN_DIM, NPAD), np.float32)
        xT[:, :NP] = xs.T
        in_maps.append(dict(
            xT=xT,
            encW=np.asarray(encW, np.float32),
            encB=np.asarray(encB, np.float32),
            thW=np.asarray(thetaW, np.float32),
            thB=np.asarray(thetaB, np.float32),
            lnG=np.asarray(lnG, np.float32),
            lnB=np.asarray(lnB, np.float32),
            linW=np.asarray(linW, np.float32),
            linB=np.asarray(linB, np.float32),
            idxA=p.idxA_w[c],
            rA=p.rA_m[c],
            idxB=p.idxB_w[c],
            rB=p.rB_m[c],
            dvc=p.dvc[c],
            dec=p.dec,
        ))

    res = run_bass_kernel_spmd(nc, in_maps, core_ids=list(range(C)), trace=trace, **runkw)
    out = np.concatenate([res.results[c]["out"] for c in range(C)], axis=0)
    return out, res, nc, p


# hardcoded problem configuration (nn_DeeperHNN_88295937671288)
_N, _E, _NNZ = 100000, 20000, 800000
_C = 8

_nc_cache = None


def kernel(x, vidx, eidx, encW, encB, thetaW, thetaB, lnG, lnB, linW, linB):
    global _nc_cache
    out, res, nc, p = run_full(
        x, vidx, eidx, encW, encB, thetaW, thetaB, lnG, lnB, linW, linB,
        N=_N, E=_E, C=_C, nc_cache=None,
    )
    _nc_cache = nc
    return out.astype(np.float32)



# revision 12
# speedup vs baseline: 2.1072x; 2.1072x over previous
"""DeeperHNN hypergraph message passing kernel for 8 Trainium2 NeuronCores.

Strategy (sharding_hint): nodes (and incidence entries, partitioned by vertex)
are sharded across 8 cores; hyperedge aggregates are computed as per-core
partials and AllReduced (replicated) since E << N; weight matrices replicated.

v2 design notes (vs the first working version):
  * One dma_gather call per 128-block (<=1024 indices), rotated across the 4
    SWDGE queues, G-pool bufs=5 so several gathers are in flight.
  * Pad tokens use index -1 (skipped by the gather ucode: no descriptors) and
    one-hot row id -1 (builds an all-zero selector row), so garbage SBUF data
    from skipped descriptors never contributes.
  * Host-side bin packing: edges are renumbered globally and nodes renumbered
    per-core so per-(block, core) token counts are balanced -> fewer slots.
  * All PE matmuls in bf16 (fp32 is 4 cycles/row).  hT is SBUF-resident.
  * LN gain/bias/relu folded into the feature-major psum-read activation after
    the PE transpose (per-partition scale/bias).
  * T_{i+1} = h_{i+1} @ thetaW computed inside layer i's phase-B block loop;
    the final linear layer likewise inside layer 3's loop.
  * The Ye AllReduce is split into 4 edge-range chunks overlapping phase A.
"""

import numpy as np

import concourse.bacc as bacc
import concourse.bass as bass
import concourse.mybir as mybir
import concourse.tile as tile
from concourse.bass_utils import run_bass_kernel_spmd
from concourse.masks import make_identity

import ml_dtypes

P = 128
BF16_NP = ml_dtypes.bfloat16
F32 = mybir.dt.float32
BF16 = mybir.dt.bfloat16
I16 = mybir.dt.int16
I32 = mybir.dt.int32
AF = mybir.ActivationFunctionType
ALU = mybir.AluOpType


PAD_IDX = 0  # pad token row: -1 skips descriptors, >=0 gathers a live row


def _cdiv(a, b):
    return (a + b - 1) // b


# ----------------------------------------------------------------------------
# Host-side preprocessing: balanced block packing + token tables.
# ----------------------------------------------------------------------------
class Prep:
    pass


def _pack_first_fit(sizes, order, nblocks, item_cap, load_cap):
    """First-fit-decreasing of items into blocks with an item-count cap and a
    load cap per block; items that fit nowhere go to the min-load block.
    sizes: [n] (or [n, C] for multi-dim loads). Returns (block, pos) per item."""
    n = len(order)
    multi = sizes.ndim == 2
    C = sizes.shape[1] if multi else 1
    loads = np.zeros((nblocks, C), np.int64)
    nitem = np.zeros(nblocks, np.int64)
    blk = np.empty(n, np.int64)
    pos = np.empty(n, np.int64)
    for i in order:
        d = sizes[i] if multi else sizes[i : i + 1]
        ok = nitem < item_cap
        fits = ok & ((loads + d) <= load_cap).all(axis=1)
        if fits.any():
            b = int(np.argmax(fits))
        else:
            m = np.where(ok[:, None], loads + d, 1 << 60).max(axis=1)
            b = int(np.argmin(m))
        blk[i] = b
        pos[i] = nitem[b]
        nitem[b] += 1
        loads[b] += d
    return blk, pos


def host_prep(vidx, eidx, N, E, C):
    p = Prep()
    NP = N // C
    NBV = _cdiv(NP, P)
    NBE = _cdiv(E, P)
    NPAD = NBV * P
    EPAD = NBE * P
    p.N, p.E, p.C, p.NP, p.NBE, p.NBV, p.NPAD, p.EPAD = N, E, C, NP, NBE, NBV, NPAD, EPAD

    vidx = np.asarray(vidx).astype(np.int64)
    eidx = np.asarray(eidx).astype(np.int64)
    de = np.bincount(eidx, minlength=E)
    dv = np.bincount(vidx, minlength=N)
    de_inv = (1.0 / np.maximum(de, 1.0)).astype(np.float32)
    dv_inv = (1.0 / np.maximum(dv, 1.0)).astype(np.float32)
    core = vidx // NP

    # ---- global edge renumbering: balance per-core counts per edge block ----
    cnt_ce = np.bincount(core * E + eidx, minlength=C * E).reshape(C, E)
    eorder = np.argsort(-de, kind="stable")
    eblk, epos = _pack_first_fit(cnt_ce.T, eorder, NBE, P, 5 * P)
    enew = eblk * P + epos  # orig edge id -> new edge id
    dec = np.zeros(EPAD, np.float32)
    dec[enew] = de_inv
    p.dec = dec.reshape(NBE, P).T.copy()

    # ---- per-core node renumbering: balance token counts per node block ----
    nodemap = np.empty((C, NP), np.int64)  # orig local id -> new local id
    invnode = np.full((C, NPAD), -1, np.int64)  # new local id -> orig local id
    dvc = np.zeros((C, P, NBV), np.float32)
    for c in range(C):
        dl = dv[c * NP : (c + 1) * NP]
        norder = np.argsort(-dl, kind="stable")
        nblk, npos = _pack_first_fit(dl.astype(np.int64), norder, NBV, P, 8 * P)
        nn = nblk * P + npos
        nodemap[c] = nn
        invnode[c, nn] = np.arange(NP)
        col = np.zeros(NPAD, np.float32)
        col[nn] = dv_inv[c * NP : (c + 1) * NP]
        dvc[c] = col.reshape(NBV, P).T
    p.nodemap = nodemap
    p.invnode = invnode
    p.dvc = dvc

    # ---- phase A tokens: per core, sorted by new edge id ----
    cntA = np.zeros((C, NBE), np.int64)
    A_ev, A_lv = [], []
    for c in range(C):
        m = core == c
        ev = enew[eidx[m]]
        lv = nodemap[c, vidx[m] - c * NP]
        o = np.argsort(ev, kind="stable")
        ev, lv = ev[o], lv[o]
        cntA[c] = np.bincount(ev // P, minlength=NBE)
        A_ev.append(ev)
        A_lv.append(lv)
    slotsA = np.maximum(1, _cdiv(cntA.max(0), P)).astype(np.int64)
    SA = int(slotsA.sum())
    TA = SA * P
    offA = np.zeros(NBE + 1, np.int64)
    np.cumsum(slotsA * P, out=offA[1:])

    idxA = np.full((C, TA), PAD_IDX, np.int16)
    rA = np.full((C, TA), -1.0, np.float32)
    for c in range(C):
        ev, lv = A_ev[c], A_lv[c]
        blk = ev // P
        starts = np.searchsorted(ev, np.arange(NBE) * P)
        tok = offA[blk] + (np.arange(len(ev)) - starts[blk])
        idxA[c, tok] = lv
        rA[c, tok] = ev - blk * P

    # ---- phase B tokens: per core, sorted by new local node id ----
    cntB = np.zeros((C, NBV), np.int64)
    B_ee, B_lv = [], []
    for c in range(C):
        m = core == c
        lv = nodemap[c, vidx[m] - c * NP]
        ee = enew[eidx[m]]
        o = np.argsort(lv, kind="stable")
        lv, ee = lv[o], ee[o]
        cntB[c] = np.bincount(lv // P, minlength=NBV)
        B_ee.append(ee)
        B_lv.append(lv)
    slotsB = np.maximum(1, _cdiv(cntB.max(0), P)).astype(np.int64)
    SB = int(slotsB.sum())
    TB = SB * P
    offB = np.zeros(NBV + 1, np.int64)
    np.cumsum(slotsB * P, out=offB[1:])

    idxB = np.full((C, TB), PAD_IDX, np.int16)
    rB = np.full((C, TB), -1.0, np.float32)
    for c in range(C):
        ee, lv = B_ee[c], B_lv[c]
        blk = lv // P
        starts = np.searchsorted(lv, np.arange(NBV) * P)
        tok = offB[blk] + (np.arange(len(lv)) - starts[blk])
        idxB[c, tok] = ee
        rB[c, tok] = lv - blk * P

    p.slotsA, p.slotsB, p.SA, p.SB, p.TA, p.TB = slotsA, slotsB, SA, SB, TA, TB
    p.offA, p.offB = offA, offB
    p.idxA_w = np.ascontiguousarray(
        np.tile(idxA.reshape(C, TA // 16, 16).transpose(0, 2, 1), (1, 8, 1))
    )
    p.rA_m = np.ascontiguousarray(rA.reshape(C, SA, P).transpose(0, 2, 1)).astype(BF16_NP)
    p.idxB_w = np.ascontiguousarray(
        np.tile(idxB.reshape(C, TB // 16, 16).transpose(0, 2, 1), (1, 8, 1))
    )
    p.rB_m = np.ascontiguousarray(rB.reshape(C, SB, P).transpose(0, 2, 1)).astype(BF16_NP)
    p.MAXSLOT = int(max(slotsA.max(), slotsB.max()))
    return p


# ----------------------------------------------------------------------------
# Device program
# ----------------------------------------------------------------------------
def build_program(p, IN_DIM, H, OUT, L, stage=99):
    C, NBE, NBV, NPAD, EPAD = p.C, p.NBE, p.NBV, p.NPAD, p.EPAD
    KI = IN_DIM // P
    KH = H // P
    assert IN_DIM % P == 0 and H % P == 0
    MS = p.MAXSLOT

    nc = bacc.Bacc(
        "TRN2",
        target_bir_lowering=False,
        debug=False,
        enable_asserts=False,
        num_devices=C,
        num_swdge_queues=4,
    )

    # ---- I/O ----
    xT_d = nc.dram_tensor("xT", [IN_DIM, NPAD], BF16, kind="ExternalInput")
    encW_d = nc.dram_tensor("encW", [IN_DIM, H], BF16, kind="ExternalInput")
    encB_d = nc.dram_tensor("encB", [H], F32, kind="ExternalInput")
    thW_d = nc.dram_tensor("thW", [L, H, H], BF16, kind="ExternalInput")
    thB_d = nc.dram_tensor("thB", [L, H], BF16, kind="ExternalInput")
    lnG_d = nc.dram_tensor("lnG", [L, H], F32, kind="ExternalInput")
    lnB_d = nc.dram_tensor("lnB", [L, H], F32, kind="ExternalInput")
    linW_d = nc.dram_tensor("linW", [H, OUT], BF16, kind="ExternalInput")
    linB_d = nc.dram_tensor("linB", [OUT], BF16, kind="ExternalInput")
    idxA_d = nc.dram_tensor("idxA", [P, p.TA // 16], I16, kind="ExternalInput")
    rA_d = nc.dram_tensor("rA", [P, p.SA], BF16, kind="ExternalInput")
    idxB_d = nc.dram_tensor("idxB", [P, p.TB // 16], I16, kind="ExternalInput")
    rB_d = nc.dram_tensor("rB", [P, p.SB], BF16, kind="ExternalInput")
    dv_d = nc.dram_tensor("dvc", [P, NBV], F32, kind="ExternalInput")
    dec_d = nc.dram_tensor("dec", [P, NBE], F32, kind="ExternalInput")
    out_d = nc.dram_tensor("out", [NPAD, OUT], F32, kind="ExternalOutput")

    # ---- internals ----
    T_d = nc.dram_tensor("T_t", [NPAD, H], BF16)
    YeP_d = nc.dram_tensor("YeP", [EPAD, H], BF16)
    YeF_d = nc.dram_tensor("YeF", [EPAD, H], BF16, addr_space="Shared")
    h_d = nc.dram_tensor("h_t", [NPAD, H], F32)

    # AllReduce chunk boundaries in edge-block units
    AR_BLKS = [0, 40, 80, 120, NBE]

    from contextlib import ExitStack
    with tile.TileContext(nc) as tc, ExitStack() as es:
        const = es.enter_context(tc.tile_pool(name="const", bufs=1))
        meta = es.enter_context(tc.tile_pool(name="meta", bufs=1))
        hTp = es.enter_context(tc.tile_pool(name="hTp", bufs=1))
        gpool = es.enter_context(tc.tile_pool(name="gpool", bufs=5))
        spool = es.enter_context(tc.tile_pool(name="spool", bufs=4))
        wrk = es.enter_context(tc.tile_pool(name="wrk", bufs=3))
        stat = es.enter_context(tc.tile_pool(name="stat", bufs=4))
        opool = es.enter_context(tc.tile_pool(name="opool", bufs=3))
        psA = es.enter_context(tc.tile_pool(name="psA", bufs=3, space="PSUM"))
        psT = es.enter_context(tc.tile_pool(name="psT", bufs=2, space="PSUM"))
        psR = es.enter_context(tc.tile_pool(name="psR", bufs=2, space="PSUM"))
        psE = es.enter_context(tc.tile_pool(name="psE", bufs=1, space="PSUM"))

        # ---- constants ----
        iota_i = const.tile([P, MS, P], I32)
        nc.gpsimd.iota(iota_i[:, :, :], pattern=[[0, MS], [1, P]], base=0,
                       channel_multiplier=0)
        iota_f = const.tile([P, MS, P], BF16)
        nc.vector.tensor_copy(iota_f[:, :, :], iota_i[:, :, :])
        ident = const.tile([P, P], BF16)
        make_identity(nc, ident[:, :])
        ones1 = const.tile([1, P], BF16)
        nc.vector.memset(ones1[:, :], 1.0)
        epsc = const.tile([P, 1], F32)
        nc.vector.memset(epsc[:, :], 1e-5)

        # weights (bf16 on-chip)
        encW_t = []
        for k in range(KI):
            row = []
            for m in range(KH):
                t = const.tile([P, P], BF16, tag=f"encW{k}{m}")
                nc.sync.dma_start(t[:, :], encW_d[k * P:(k + 1) * P, m * P:(m + 1) * P])
                row.append(t)
            encW_t.append(row)
        encB_c = []
        for m in range(KH):
            t = const.tile([P, 1], F32, tag=f"encB{m}")
            nc.sync.dma_start(t[:, :], encB_d[m * P:(m + 1) * P, None])
            encB_c.append(t)
        thW_t = []
        for i in range(L):
            row = []
            for k in range(KH):
                t = const.tile([P, H], BF16, tag=f"thW{i}{k}")
                nc.sync.dma_start(t[:, :], thW_d[i, k * P:(k + 1) * P, :])
                row.append(t)
            thW_t.append(row)
        thB_t = []
        for i in range(L):
            t = const.tile([1, H], BF16, tag=f"thB{i}")
            nc.sync.dma_start(t[:, :], thB_d[i:i + 1, :])
            thB_t.append(t)
        linW_t = []
        for k in range(KH):
            t = const.tile([P, OUT], BF16, tag=f"linW{k}")
            nc.sync.dma_start(t[:, :], linW_d[k * P:(k + 1) * P, :])
            linW_t.append(t)
        linB_t = const.tile([1, OUT], BF16)
        nc.sync.dma_start(linB_t[:, :], linB_d[None, :])
        # LN gain/bias as feature-major per-partition columns
        lnG_c, lnB_c = [], []
        for i in range(L):
            gs, bs = [], []
            for m in range(KH):
                g = const.tile([P, 1], F32, tag=f"lnG{i}{m}")
                b = const.tile([P, 1], F32, tag=f"lnB{i}{m}")
                nc.sync.dma_start(g[:, :], lnG_d[i, m * P:(m + 1) * P, None])
                nc.sync.dma_start(b[:, :], lnB_d[i, m * P:(m + 1) * P, None])
                gs.append(g)
                bs.append(b)
            lnG_c.append(gs)
            lnB_c.append(bs)

        # metadata
        idxA_t = meta.tile([P, p.TA // 16], I16)
        nc.sync.dma_start(idxA_t[:, :], idxA_d[:, :])
        rA_t = meta.tile([P, p.SA], BF16)
        nc.sync.dma_start(rA_t[:, :], rA_d[:, :])
        dec_t = meta.tile([P, NBE], F32)
        nc.sync.dma_start(dec_t[:, :], dec_d[:, :])
        idxB_t = meta.tile([P, p.TB // 16], I16)
        nc.sync.dma_start(idxB_t[:, :], idxB_d[:, :])
        rB_t = meta.tile([P, p.SB], BF16)
        nc.sync.dma_start(rB_t[:, :], rB_d[:, :])
        dv_t = meta.tile([P, NBV], F32)
        nc.sync.dma_start(dv_t[:, :], dv_d[:, :])

        # resident transposed activations hT [feat, nodes] (bf16), one tile
        # per node block so each region has a single writer
        hT_b = [hTp.tile([P, KH, P], BF16, tag=f"hT{vb}", name=f"hT{vb}")
                for vb in range(NBV)]

        qn = [0]

        def next_q():
            q = qn[0]
            qn[0] = (q + 1) % 4
            return q

        def emit_gather(G, src_d, idx_t, s0, sb):
            g0 = 0
            while g0 < sb:
                gs = min(8, sb - g0)
                tok0 = (s0 + g0) * P
                nc.gpsimd.dma_gather(
                    out_ap=G[:, g0:g0 + gs, :],
                    in_ap=src_d[:, :],
                    idxs_ap=idx_t[:, tok0 // 16:(tok0 + gs * P) // 16],
                    num_idxs=gs * P,
                    num_idxs_reg=gs * P,
                    elem_size=H,
                    queue_num=next_q(),
                )
                g0 += gs

        def emit_T_block(li, vb):
            """T_d[vb block] = t @ thetaW[li] + thetaB[li], t from resident hT."""
            ps = psT.tile([P, H], F32, tag="psT")
            for k in range(KH):
                nc.tensor.matmul(ps[:, :], lhsT=hT_b[vb][:, k, :],
                                 rhs=thW_t[li][k][:, :],
                                 start=(k == 0), stop=False)
            nc.tensor.matmul(ps[:, :], lhsT=ones1[:1, :], rhs=thB_t[li][:1, :],
                             start=False, stop=True)
            Tb = opool.tile([P, H], BF16, tag="Tout")
            nc.scalar.activation(Tb[:, :], ps[:, :], AF.Copy)
            nc.sync.dma_start(T_d[vb * P:(vb + 1) * P, :], Tb[:, :])

        def emit_final_block(vb):
            """out_d[vb block] = t @ linW + linB, t from resident hT."""
            ps = psT.tile([P, H], F32, tag="psT")
            for k in range(KH):
                nc.tensor.matmul(ps[:, :OUT], lhsT=hT_b[vb][:, k, :],
                                 rhs=linW_t[k][:, :],
                                 start=(k == 0), stop=False)
            nc.tensor.matmul(ps[:, :OUT], lhsT=ones1[:1, :], rhs=linB_t[:1, :],
                             start=False, stop=True)
            ob = opool.tile([P, OUT], F32, tag="finout")
            nc.scalar.activation(ob[:, :], ps[:, :OUT], AF.Copy)
            nc.sync.dma_start(out_d[vb * P:(vb + 1) * P, :], ob[:, :])

        # ------------------------------------------------------------------
        # Encoder: hT = (x @ encW + encB)^T directly feature-major, and
        # layer-0 T blocks as soon as their hT columns exist.
        # ------------------------------------------------------------------
        CW = 512
        for c0 in range(0, NPAD, CW):
            ncols = min(CW, NPAD - c0)
            xc = wrk.tile([P, KI, CW], BF16, tag="xc")
            nc.sync.dma_start(
                xc[:, :, :ncols],
                xT_d.ap().rearrange("(k q) n -> q k n", q=P)[:, :, c0:c0 + ncols],
            )
            for m in range(KH):
                ps = psE.tile([P, CW], F32, tag="psE")
                for k in range(KI):
                    nc.tensor.matmul(ps[:, :ncols], lhsT=encW_t[k][m][:, :],
                                     rhs=xc[:, k, :ncols],
                                     start=(k == 0), stop=(k == KI - 1))
                for j in range(ncols // P):
                    nc.scalar.activation(hT_b[c0 // P + j][:, m, :],
                                         ps[:, j * P:(j + 1) * P],
                                         AF.Identity, bias=encB_c[m][:, :], scale=1.0)
            if stage >= 2:
                for vb in range(c0 // P, (c0 + ncols) // P):
                    emit_T_block(0, vb)

        # ------------------------------------------------------------------
        # Conv layers
        # ------------------------------------------------------------------
        for li in range(L if stage >= 3 else 0):
            # ---- Phase A: partial Ye, one gather call per edge block ----
            ar_next = 1
            for eb in range(NBE):
                sb = int(p.slotsA[eb])
                s0 = int(p.offA[eb]) // P
                G = gpool.tile([P, MS, H], BF16, tag="G")
                emit_gather(G, T_d, idxA_t, s0, sb)
                S = spool.tile([P, MS, P], BF16, tag="S")
                rb_ap = rA_t[:, s0:s0 + sb].unsqueeze(2).broadcast_to([P, sb, P])
                nc.vector.tensor_tensor(S[:, :sb, :], iota_f[:, :sb, :], rb_ap,
                                        op=ALU.is_equal)
                ps = psA.tile([P, H], F32, tag="psA")
                for s in range(sb):
                    nc.tensor.matmul(ps[:, :], lhsT=S[:, s, :], rhs=G[:, s, :],
                                     start=(s == 0), stop=(s == sb - 1))
                yeb = opool.tile([P, H], BF16, tag="yeg")
                nc.scalar.activation(yeb[:, :], ps[:, :], AF.Copy,
                                     scale=dec_t[:, eb:eb + 1])
                nc.scalar.dma_start(YeP_d[eb * P:(eb + 1) * P, :], yeb[:, :])
                if stage >= 4 and ar_next < len(AR_BLKS) and eb == AR_BLKS[ar_next] - 1:
                    r0, r1 = AR_BLKS[ar_next - 1] * P, AR_BLKS[ar_next] * P
                    nc.gpsimd.collective_compute(
                        "AllReduce",
                        ALU.add,
                        replica_groups=[list(range(C))],
                        ins=[YeP_d.ap()[r0:r1, :]],
                        outs=[YeF_d.ap()[r0:r1, :]],
                    )
                    ar_next += 1

            if stage < 5:
                continue

            # ---- Phase B: conv + residual + LN tail + next-layer T ----
            lnxt = li + 1 if li + 1 < L else 0
            for vb in range(NBV):
                sb = int(p.slotsB[vb])
                s0 = int(p.offB[vb]) // P
                G = gpool.tile([P, MS, H], BF16, tag="G")
                emit_gather(G, YeF_d, idxB_t, s0, sb)
                S = spool.tile([P, MS, P], BF16, tag="S")
                rb_ap = rB_t[:, s0:s0 + sb].unsqueeze(2).broadcast_to([P, sb, P])
                nc.vector.tensor_tensor(S[:, :sb, :], iota_f[:, :sb, :], rb_ap,
                                        op=ALU.is_equal)
                ps = psA.tile([P, H], F32, tag="psA")
                for s in range(sb):
                    nc.tensor.matmul(ps[:, :], lhsT=S[:, s, :], rhs=G[:, s, :],
                                     start=(s == 0), stop=(s == sb - 1))
                # relu(dv * x) == dv * relu(x) since dv >= 0
                hn = wrk.tile([P, H], F32, tag="hn")
                nc.scalar.activation(hn[:, :], ps[:, :], AF.Relu,
                                     scale=dv_t[:, vb:vb + 1])
                if li > 0:
                    hp = wrk.tile([P, H], F32, tag="hp")
                    nc.sync.dma_start(hp[:, :], h_d[vb * P:(vb + 1) * P, :])
                    nc.vector.tensor_add(hn[:, :], hn[:, :], hp[:, :])
                if li < L - 1:
                    nc.sync.dma_start(h_d[vb * P:(vb + 1) * P, :], hn[:, :])

                if stage < 6:
                    continue
                # LN stats + normalize (row-major), affine+relu after transpose
                st6 = stat.tile([P, 6], F32, tag="st6")
                nc.vector.bn_stats(st6[:, :], hn[:, :])
                mv = stat.tile([P, 2], F32, tag="mv")
                nc.vector.bn_aggr(mv[:, :], st6[:, :])
                rstd = stat.tile([P, 1], F32, tag="rstd")
                nc.scalar.activation(rstd[:, :], mv[:, 1:2], AF.Sqrt,
                                     bias=epsc[:, :], scale=1.0)
                rinv = stat.tile([P, 1], F32, tag="rinv")
                nc.vector.reciprocal(rinv[:, :], rstd[:, :])
                zt = wrk.tile([P, H], BF16, tag="zt")
                nc.vector.tensor_scalar(zt[:, :], hn[:, :], mv[:, 0:1], rinv[:, :],
                                        op0=ALU.subtract, op1=ALU.mult)
                for m in range(KH):
                    pst = psR.tile([P, P], BF16, tag="psR")
                    nc.tensor.transpose(pst[:, :], zt[:, m * P:(m + 1) * P], ident[:, :])
                    nc.scalar.activation(hT_b[vb][:, m, :], pst[:, :],
                                         AF.Relu, bias=lnB_c[lnxt][m][:, :],
                                         scale=lnG_c[lnxt][m][:, :])
                if li < L - 1:
                    emit_T_block(li + 1, vb)
                else:
                    emit_final_block(vb)

    nc.compile()
    return nc


# ----------------------------------------------------------------------------
# Full pipeline: prep + build + run
# ----------------------------------------------------------------------------
def run_full(x, vidx, eidx, encW, encB, thetaW, thetaB, lnG, lnB, linW, linB,
             N, E, C, trace=False, nc_cache=None, stage=99, **runkw):
    IN_DIM = x.shape[1]
    H = encW.shape[1]
    OUT = linW.shape[1]
    L = thetaW.shape[0]

    p = host_prep(np.asarray(vidx), np.asarray(eidx), N, E, C)
    nc = nc_cache if nc_cache is not None else build_program(p, IN_DIM, H, OUT, L, stage=stage)

    x = np.asarray(x, np.float32)
    NP, NPAD = p.NP, p.NPAD
    in_maps = []
    for c in range(C):
        xs = x[c * NP:(c + 1) * NP]
        xT = np.zeros((IN_DIM, NPAD), BF16_NP)
        xT[:, p.nodemap[c]] = xs.T.astype(BF16_NP)
        in_maps.append(dict(
            xT=xT,
            encW=np.asarray(encW, np.float32).astype(BF16_NP),
            encB=np.asarray(encB, np.float32),
            thW=np.asarray(thetaW, np.float32).astype(BF16_NP),
            thB=np.asarray(thetaB, np.float32).astype(BF16_NP),
            lnG=np.asarray(lnG, np.float32),
            lnB=np.asarray(lnB, np.float32),
            linW=np.asarray(linW, np.float32).astype(BF16_NP),
            linB=np.asarray(linB, np.float32).astype(BF16_NP),
            idxA=p.idxA_w[c],
            rA=p.rA_m[c],
            idxB=p.idxB_w[c],
            rB=p.rB_m[c],
            dvc=p.dvc[c],
            dec=p.dec,
        ))

    res = run_bass_kernel_spmd(nc, in_maps, core_ids=list(range(C)), trace=trace, **runkw)
    outs = []
    for c in range(C):
        op = res.results[c]["out"]  # [NPAD, OUT] in permuted order
        outs.append(op[p.nodemap[c]])
    out = np.concatenate(outs, axis=0)
    return out, res, nc, p


# hardcoded problem configuration (nn_DeeperHNN_88295937671288)
_N, _E, _NNZ = 100000, 20000, 800000
_C = 8

_nc_cache = None


def kernel(x, vidx, eidx, encW, encB, thetaW, thetaB, lnG, lnB, linW, linB):
    global _nc_cache
    out, res, nc, p = run_full(
        x, vidx, eidx, encW, encB, thetaW, thetaB, lnG, lnB, linW, linB,
        N=_N, E=_E, C=_C, nc_cache=None,
    )
    _nc_cache = nc
    return out.astype(np.float32)


# revision 15
# speedup vs baseline: 2.5479x; 1.2092x over previous
"""DeeperHNN hypergraph message passing kernel for 8 Trainium2 NeuronCores.

Strategy (sharding_hint): nodes (and incidence entries, partitioned by vertex)
are sharded across 8 cores; hyperedge aggregates are computed as per-core
partials and AllReduced (replicated) since E << N; weight matrices replicated.

v2 design notes (vs the first working version):
  * One dma_gather call per 128-block (<=1024 indices), rotated across the 4
    SWDGE queues, G-pool bufs=5 so several gathers are in flight.
  * Pad tokens use index -1 (skipped by the gather ucode: no descriptors) and
    one-hot row id -1 (builds an all-zero selector row), so garbage SBUF data
    from skipped descriptors never contributes.
  * Host-side bin packing: edges are renumbered globally and nodes renumbered
    per-core so per-(block, core) token counts are balanced -> fewer slots.
  * All PE matmuls in bf16 (fp32 is 4 cycles/row).  hT is SBUF-resident.
  * LN gain/bias/relu folded into the feature-major psum-read activation after
    the PE transpose (per-partition scale/bias).
  * T_{i+1} = h_{i+1} @ thetaW computed inside layer i's phase-B block loop;
    the final linear layer likewise inside layer 3's loop.
  * The Ye AllReduce is split into 4 edge-range chunks overlapping phase A.
"""

import numpy as np

import concourse.bacc as bacc
import concourse.bass as bass
import concourse.mybir as mybir
import concourse.tile as tile
from concourse.bass_utils import run_bass_kernel_spmd
from concourse.masks import make_identity

import ml_dtypes

P = 128
BF16_NP = ml_dtypes.bfloat16
F32 = mybir.dt.float32
BF16 = mybir.dt.bfloat16
I16 = mybir.dt.int16
I32 = mybir.dt.int32
AF = mybir.ActivationFunctionType
ALU = mybir.AluOpType


PAD_IDX = 0  # pad token row: -1 skips descriptors, >=0 gathers a live row


def _cdiv(a, b):
    return (a + b - 1) // b


# ----------------------------------------------------------------------------
# Host-side preprocessing: balanced block packing + token tables.
# ----------------------------------------------------------------------------
class Prep:
    pass


def _pack_first_fit(sizes, order, nblocks, item_cap, load_cap):
    """First-fit-decreasing of items into blocks with an item-count cap and a
    load cap per block; items that fit nowhere go to the min-load block.
    sizes: [n] (or [n, C] for multi-dim loads). Returns (block, pos) per item."""
    n = len(order)
    multi = sizes.ndim == 2
    C = sizes.shape[1] if multi else 1
    loads = np.zeros((nblocks, C), np.int64)
    nitem = np.zeros(nblocks, np.int64)
    blk = np.empty(n, np.int64)
    pos = np.empty(n, np.int64)
    for i in order:
        d = sizes[i] if multi else sizes[i : i + 1]
        ok = nitem < item_cap
        fits = ok & ((loads + d) <= load_cap).all(axis=1)
        if fits.any():
            b = int(np.argmax(fits))
        else:
            m = np.where(ok[:, None], loads + d, 1 << 60).max(axis=1)
            b = int(np.argmin(m))
        blk[i] = b
        pos[i] = nitem[b]
        nitem[b] += 1
        loads[b] += d
    return blk, pos


def host_prep(vidx, eidx, N, E, C):
    p = Prep()
    NP = N // C
    NBV = _cdiv(NP, P)
    NBE = _cdiv(E, P)
    NPAD = NBV * P
    EPAD = NBE * P
    p.N, p.E, p.C, p.NP, p.NBE, p.NBV, p.NPAD, p.EPAD = N, E, C, NP, NBE, NBV, NPAD, EPAD

    vidx = np.asarray(vidx).astype(np.int64)
    eidx = np.asarray(eidx).astype(np.int64)
    de = np.bincount(eidx, minlength=E)
    dv = np.bincount(vidx, minlength=N)
    de_inv = (1.0 / np.maximum(de, 1.0)).astype(np.float32)
    dv_inv = (1.0 / np.maximum(dv, 1.0)).astype(np.float32)
    core = vidx // NP

    # ---- global edge renumbering: balance per-core counts per edge block ----
    cnt_ce = np.bincount(core * E + eidx, minlength=C * E).reshape(C, E)
    eorder = np.argsort(-de, kind="stable")
    eblk, epos = _pack_first_fit(cnt_ce.T, eorder, NBE, P, 5 * P)
    enew = eblk * P + epos  # orig edge id -> new edge id
    dec = np.zeros(EPAD, np.float32)
    dec[enew] = de_inv
    p.dec = dec.reshape(NBE, P).T.copy()

    # ---- per-core node renumbering: balance token counts per node block ----
    nodemap = np.empty((C, NP), np.int64)  # orig local id -> new local id
    invnode = np.full((C, NPAD), -1, np.int64)  # new local id -> orig local id
    dvc = np.zeros((C, P, NBV), np.float32)
    for c in range(C):
        dl = dv[c * NP : (c + 1) * NP]
        norder = np.argsort(-dl, kind="stable")
        nblk, npos = _pack_first_fit(dl.astype(np.int64), norder, NBV, P, 8 * P)
        nn = nblk * P + npos
        nodemap[c] = nn
        invnode[c, nn] = np.arange(NP)
        col = np.zeros(NPAD, np.float32)
        col[nn] = dv_inv[c * NP : (c + 1) * NP]
        dvc[c] = col.reshape(NBV, P).T
    p.nodemap = nodemap
    p.invnode = invnode
    p.dvc = dvc

    # ---- phase A tokens: per core, sorted by new edge id ----
    cntA = np.zeros((C, NBE), np.int64)
    A_ev, A_lv = [], []
    for c in range(C):
        m = core == c
        ev = enew[eidx[m]]
        lv = nodemap[c, vidx[m] - c * NP]
        o = np.argsort(ev, kind="stable")
        ev, lv = ev[o], lv[o]
        cntA[c] = np.bincount(ev // P, minlength=NBE)
        A_ev.append(ev)
        A_lv.append(lv)
    slotsA = np.maximum(1, _cdiv(cntA.max(0), P)).astype(np.int64)
    SA = int(slotsA.sum())
    TA = SA * P
    offA = np.zeros(NBE + 1, np.int64)
    np.cumsum(slotsA * P, out=offA[1:])

    idxA = np.full((C, TA), PAD_IDX, np.int16)
    rA = np.full((C, TA), -1.0, np.float32)
    for c in range(C):
        ev, lv = A_ev[c], A_lv[c]
        blk = ev // P
        starts = np.searchsorted(ev, np.arange(NBE) * P)
        tok = offA[blk] + (np.arange(len(ev)) - starts[blk])
        idxA[c, tok] = lv
        rA[c, tok] = ev - blk * P

    # ---- phase B tokens: per core, sorted by new local node id ----
    cntB = np.zeros((C, NBV), np.int64)
    B_ee, B_lv = [], []
    for c in range(C):
        m = core == c
        lv = nodemap[c, vidx[m] - c * NP]
        ee = enew[eidx[m]]
        o = np.argsort(lv, kind="stable")
        lv, ee = lv[o], ee[o]
        cntB[c] = np.bincount(lv // P, minlength=NBV)
        B_ee.append(ee)
        B_lv.append(lv)
    slotsB = np.maximum(1, _cdiv(cntB.max(0), P)).astype(np.int64)
    SB = int(slotsB.sum())
    TB = SB * P
    offB = np.zeros(NBV + 1, np.int64)
    np.cumsum(slotsB * P, out=offB[1:])

    idxB = np.full((C, TB), PAD_IDX, np.int16)
    rB = np.full((C, TB), -1.0, np.float32)
    for c in range(C):
        ee, lv = B_ee[c], B_lv[c]
        blk = lv // P
        starts = np.searchsorted(lv, np.arange(NBV) * P)
        tok = offB[blk] + (np.arange(len(lv)) - starts[blk])
        idxB[c, tok] = ee
        rB[c, tok] = lv - blk * P

    p.slotsA, p.slotsB, p.SA, p.SB, p.TA, p.TB = slotsA, slotsB, SA, SB, TA, TB
    p.offA, p.offB = offA, offB
    p.idxA_w = np.ascontiguousarray(
        np.tile(idxA.reshape(C, TA // 16, 16).transpose(0, 2, 1), (1, 8, 1))
    )
    p.rA_m = np.ascontiguousarray(rA.reshape(C, SA, P).transpose(0, 2, 1)).astype(BF16_NP)
    p.idxB_w = np.ascontiguousarray(
        np.tile(idxB.reshape(C, TB // 16, 16).transpose(0, 2, 1), (1, 8, 1))
    )
    p.rB_m = np.ascontiguousarray(rB.reshape(C, SB, P).transpose(0, 2, 1)).astype(BF16_NP)
    p.MAXSLOT = int(max(slotsA.max(), slotsB.max()))
    return p


# ----------------------------------------------------------------------------
# Device program
# ----------------------------------------------------------------------------
def build_program(p, IN_DIM, H, OUT, L, stage=99):
    C, NBE, NBV, NPAD, EPAD = p.C, p.NBE, p.NBV, p.NPAD, p.EPAD
    KI = IN_DIM // P
    KH = H // P
    assert IN_DIM % P == 0 and H % P == 0
    MS = p.MAXSLOT
    pairA = [int(p.slotsA[i] + (p.slotsA[i + 1] if i + 1 < NBE else 0))
             for i in range(0, NBE, 2)]
    GMS = max(max(pairA), int(p.slotsB.max()))

    nc = bacc.Bacc(
        "TRN2",
        target_bir_lowering=False,
        debug=False,
        enable_asserts=False,
        num_devices=C,
        num_swdge_queues=4,
    )

    # ---- I/O ----
    xT_d = nc.dram_tensor("xT", [IN_DIM, NPAD], BF16, kind="ExternalInput")
    encW_d = nc.dram_tensor("encW", [IN_DIM, H], BF16, kind="ExternalInput")
    encB_d = nc.dram_tensor("encB", [H], F32, kind="ExternalInput")
    thW_d = nc.dram_tensor("thW", [L, H, H], BF16, kind="ExternalInput")
    thB_d = nc.dram_tensor("thB", [L, H], BF16, kind="ExternalInput")
    lnG_d = nc.dram_tensor("lnG", [L, H], F32, kind="ExternalInput")
    lnB_d = nc.dram_tensor("lnB", [L, H], F32, kind="ExternalInput")
    linW_d = nc.dram_tensor("linW", [H, OUT], BF16, kind="ExternalInput")
    linB_d = nc.dram_tensor("linB", [OUT], BF16, kind="ExternalInput")
    idxA_d = nc.dram_tensor("idxA", [P, p.TA // 16], I16, kind="ExternalInput")
    rA_d = nc.dram_tensor("rA", [P, p.SA], BF16, kind="ExternalInput")
    idxB_d = nc.dram_tensor("idxB", [P, p.TB // 16], I16, kind="ExternalInput")
    rB_d = nc.dram_tensor("rB", [P, p.SB], BF16, kind="ExternalInput")
    dv_d = nc.dram_tensor("dvc", [P, NBV], F32, kind="ExternalInput")
    dec_d = nc.dram_tensor("dec", [P, NBE], F32, kind="ExternalInput")
    out_d = nc.dram_tensor("out", [NPAD, OUT], F32, kind="ExternalOutput")

    # ---- internals ----
    T_d = nc.dram_tensor("T_t", [NPAD, H], BF16)
    YeP_d = nc.dram_tensor("YeP", [EPAD, H], BF16)
    YeF_d = nc.dram_tensor("YeF", [EPAD, H], BF16, addr_space="Shared")
    h_d = nc.dram_tensor("h_t", [NPAD, H], F32)

    # AllReduce chunk boundaries in edge-block units
    AR_BLKS = [0, 40, 80, 120, NBE]

    from contextlib import ExitStack
    with tile.TileContext(nc) as tc, ExitStack() as es:
        const = es.enter_context(tc.tile_pool(name="const", bufs=1))
        meta = es.enter_context(tc.tile_pool(name="meta", bufs=1))
        hTp = es.enter_context(tc.tile_pool(name="hTp", bufs=1))
        gpool = es.enter_context(tc.tile_pool(name="gpool", bufs=5))
        spool = es.enter_context(tc.tile_pool(name="spool", bufs=4))
        wrk = es.enter_context(tc.tile_pool(name="wrk", bufs=3))
        stat = es.enter_context(tc.tile_pool(name="stat", bufs=4))
        opool = es.enter_context(tc.tile_pool(name="opool", bufs=3))
        psA = es.enter_context(tc.tile_pool(name="psA", bufs=3, space="PSUM"))
        psT = es.enter_context(tc.tile_pool(name="psT", bufs=2, space="PSUM"))
        psR = es.enter_context(tc.tile_pool(name="psR", bufs=2, space="PSUM"))
        psE = es.enter_context(tc.tile_pool(name="psE", bufs=1, space="PSUM"))

        # ---- constants ----
        iota_i = const.tile([P, P], I32)
        nc.gpsimd.iota(iota_i[:, :], pattern=[[1, P]], base=0,
                       channel_multiplier=0)
        iota_f = const.tile([P, P], BF16)
        nc.vector.tensor_copy(iota_f[:, :], iota_i[:, :])
        ident = const.tile([P, P], BF16)
        make_identity(nc, ident[:, :])
        ones1 = const.tile([1, P], BF16)
        nc.vector.memset(ones1[:, :], 1.0)
        epsc = const.tile([P, 1], F32)
        nc.vector.memset(epsc[:, :], 1e-5)

        # weights (bf16 on-chip)
        encW_t = []
        for k in range(KI):
            row = []
            for m in range(KH):
                t = const.tile([P, P], BF16, tag=f"encW{k}{m}")
                nc.sync.dma_start(t[:, :], encW_d[k * P:(k + 1) * P, m * P:(m + 1) * P])
                row.append(t)
            encW_t.append(row)
        encB_c = []
        for m in range(KH):
            t = const.tile([P, 1], F32, tag=f"encB{m}")
            nc.sync.dma_start(t[:, :], encB_d[m * P:(m + 1) * P, None])
            encB_c.append(t)
        thW_t = []
        for i in range(L):
            row = []
            for k in range(KH):
                t = const.tile([P, H], BF16, tag=f"thW{i}{k}")
                nc.sync.dma_start(t[:, :], thW_d[i, k * P:(k + 1) * P, :])
                row.append(t)
            thW_t.append(row)
        thB_t = []
        for i in range(L):
            t = const.tile([1, H], BF16, tag=f"thB{i}")
            nc.sync.dma_start(t[:, :], thB_d[i:i + 1, :])
            thB_t.append(t)
        linW_t = []
        for k in range(KH):
            t = const.tile([P, OUT], BF16, tag=f"linW{k}")
            nc.sync.dma_start(t[:, :], linW_d[k * P:(k + 1) * P, :])
            linW_t.append(t)
        linB_t = const.tile([1, OUT], BF16)
        nc.sync.dma_start(linB_t[:, :], linB_d[None, :])
        # LN gain/bias as feature-major per-partition columns
        lnG_c, lnB_c = [], []
        for i in range(L):
            gs, bs = [], []
            for m in range(KH):
                g = const.tile([P, 1], F32, tag=f"lnG{i}{m}")
                b = const.tile([P, 1], F32, tag=f"lnB{i}{m}")
                nc.sync.dma_start(g[:, :], lnG_d[i, m * P:(m + 1) * P, None])
                nc.sync.dma_start(b[:, :], lnB_d[i, m * P:(m + 1) * P, None])
                gs.append(g)
                bs.append(b)
            lnG_c.append(gs)
            lnB_c.append(bs)

        # metadata
        idxA_t = meta.tile([P, p.TA // 16], I16)
        nc.sync.dma_start(idxA_t[:, :], idxA_d[:, :])
        rA_t = meta.tile([P, p.SA], BF16)
        nc.sync.dma_start(rA_t[:, :], rA_d[:, :])
        dec_t = meta.tile([P, NBE], F32)
        nc.sync.dma_start(dec_t[:, :], dec_d[:, :])
        idxB_t = meta.tile([P, p.TB // 16], I16)
        nc.sync.dma_start(idxB_t[:, :], idxB_d[:, :])
        rB_t = meta.tile([P, p.SB], BF16)
        nc.sync.dma_start(rB_t[:, :], rB_d[:, :])
        dv_t = meta.tile([P, NBV], F32)
        nc.sync.dma_start(dv_t[:, :], dv_d[:, :])

        # resident transposed activations hT [feat, nodes] (bf16), one tile
        # per node block so each region has a single writer
        hT_b = [hTp.tile([P, KH, P], BF16, tag=f"hT{vb}", name=f"hT{vb}")
                for vb in range(NBV)]

        qn = [0]

        def next_q():
            q = qn[0]
            qn[0] = (q + 1) % 4
            return q

        def emit_gather(G, src_d, idx_t, s0, sb):
            g0 = 0
            while g0 < sb:
                gs = min(8, sb - g0)
                tok0 = (s0 + g0) * P
                nc.gpsimd.dma_gather(
                    out_ap=G[:, g0:g0 + gs, :],
                    in_ap=src_d[:, :],
                    idxs_ap=idx_t[:, tok0 // 16:(tok0 + gs * P) // 16],
                    num_idxs=gs * P,
                    num_idxs_reg=gs * P,
                    elem_size=H,
                    queue_num=next_q(),
                )
                g0 += gs

        def emit_T_block(li, vb):
            """T_d[vb block] = t @ thetaW[li] + thetaB[li], t from resident hT."""
            ps = psT.tile([P, H], F32, tag="psT")
            for k in range(KH):
                nc.tensor.matmul(ps[:, :], lhsT=hT_b[vb][:, k, :],
                                 rhs=thW_t[li][k][:, :],
                                 start=(k == 0), stop=False)
            nc.tensor.matmul(ps[:, :], lhsT=ones1[:1, :], rhs=thB_t[li][:1, :],
                             start=False, stop=True)
            Tb = opool.tile([P, H], BF16, tag="Tout")
            nc.scalar.activation(Tb[:, :], ps[:, :], AF.Copy)
            nc.sync.dma_start(T_d[vb * P:(vb + 1) * P, :], Tb[:, :])

        def emit_final_block(vb):
            """out_d[vb block] = t @ linW + linB, t from resident hT."""
            ps = psT.tile([P, H], F32, tag="psT")
            for k in range(KH):
                nc.tensor.matmul(ps[:, :OUT], lhsT=hT_b[vb][:, k, :],
                                 rhs=linW_t[k][:, :],
                                 start=(k == 0), stop=False)
            nc.tensor.matmul(ps[:, :OUT], lhsT=ones1[:1, :], rhs=linB_t[:1, :],
                             start=False, stop=True)
            ob = opool.tile([P, OUT], F32, tag="finout")
            nc.scalar.activation(ob[:, :], ps[:, :OUT], AF.Copy)
            nc.sync.dma_start(out_d[vb * P:(vb + 1) * P, :], ob[:, :])

        # ------------------------------------------------------------------
        # Encoder: hT = (x @ encW + encB)^T directly feature-major, and
        # layer-0 T blocks as soon as their hT columns exist.
        # ------------------------------------------------------------------
        CW = 512
        for c0 in range(0, NPAD, CW):
            ncols = min(CW, NPAD - c0)
            xc = wrk.tile([P, KI, CW], BF16, tag="xc")
            nc.sync.dma_start(
                xc[:, :, :ncols],
                xT_d.ap().rearrange("(k q) n -> q k n", q=P)[:, :, c0:c0 + ncols],
            )
            for m in range(KH):
                ps = psE.tile([P, CW], F32, tag="psE")
                for k in range(KI):
                    nc.tensor.matmul(ps[:, :ncols], lhsT=encW_t[k][m][:, :],
                                     rhs=xc[:, k, :ncols],
                                     start=(k == 0), stop=(k == KI - 1))
                for j in range(ncols // P):
                    nc.scalar.activation(hT_b[c0 // P + j][:, m, :],
                                         ps[:, j * P:(j + 1) * P],
                                         AF.Identity, bias=encB_c[m][:, :], scale=1.0)
            if stage >= 2:
                for vb in range(c0 // P, (c0 + ncols) // P):
                    emit_T_block(0, vb)

        # ------------------------------------------------------------------
        # Conv layers
        # ------------------------------------------------------------------
        for li in range(L if stage >= 3 else 0):
            # ---- Phase A: partial Ye, one gather call per PAIR of blocks ----
            ar_next = 1
            for eb0 in range(0, NBE, 2):
                npair = min(2, NBE - eb0)
                sbs = [int(p.slotsA[eb0 + j]) for j in range(npair)]
                s0 = int(p.offA[eb0]) // P
                sbt = sum(sbs)
                G = gpool.tile([P, GMS, H], BF16, tag="G")
                emit_gather(G, T_d, idxA_t, s0, sbt)
                S = spool.tile([P, GMS, P], BF16, tag="S")
                iota_ap = iota_f[:, :].unsqueeze(1).broadcast_to([P, sbt, P])
                rb_ap = rA_t[:, s0:s0 + sbt].unsqueeze(2).broadcast_to([P, sbt, P])
                nc.vector.tensor_tensor(S[:, :sbt, :], iota_ap, rb_ap,
                                        op=ALU.is_equal)
                so = 0
                for j in range(npair):
                    eb = eb0 + j
                    sb = sbs[j]
                    ps = psA.tile([P, H], F32, tag="psA")
                    for s in range(sb):
                        nc.tensor.matmul(ps[:, :], lhsT=S[:, so + s, :],
                                         rhs=G[:, so + s, :],
                                         start=(s == 0), stop=(s == sb - 1))
                    so += sb
                    yeb = opool.tile([P, H], BF16, tag="yeg")
                    nc.scalar.activation(yeb[:, :], ps[:, :], AF.Copy,
                                         scale=dec_t[:, eb:eb + 1])
                    nc.scalar.dma_start(YeP_d[eb * P:(eb + 1) * P, :], yeb[:, :])
                eb = eb0 + npair - 1
                if stage >= 4 and ar_next < len(AR_BLKS) and eb == AR_BLKS[ar_next] - 1:
                    r0, r1 = AR_BLKS[ar_next - 1] * P, AR_BLKS[ar_next] * P
                    nc.gpsimd.collective_compute(
                        "AllReduce",
                        ALU.add,
                        replica_groups=[list(range(C))],
                        ins=[YeP_d.ap()[r0:r1, :]],
                        outs=[YeF_d.ap()[r0:r1, :]],
                    )
                    ar_next += 1

            if stage < 5:
                continue

            # ---- Phase B: conv + residual + LN tail + next-layer T ----
            lnxt = li + 1 if li + 1 < L else 0
            for vb in range(NBV):
                sb = int(p.slotsB[vb])
                s0 = int(p.offB[vb]) // P
                G = gpool.tile([P, GMS, H], BF16, tag="G")
                emit_gather(G, YeF_d, idxB_t, s0, sb)
                S = spool.tile([P, GMS, P], BF16, tag="S")
                iota_ap = iota_f[:, :].unsqueeze(1).broadcast_to([P, sb, P])
                rb_ap = rB_t[:, s0:s0 + sb].unsqueeze(2).broadcast_to([P, sb, P])
                nc.vector.tensor_tensor(S[:, :sb, :], iota_ap, rb_ap,
                                        op=ALU.is_equal)
                ps = psA.tile([P, H], F32, tag="psA")
                for s in range(sb):
                    nc.tensor.matmul(ps[:, :], lhsT=S[:, s, :], rhs=G[:, s, :],
                                     start=(s == 0), stop=(s == sb - 1))
                # relu(dv * x) == dv * relu(x) since dv >= 0
                hn = wrk.tile([P, H], F32, tag="hn")
                nc.scalar.activation(hn[:, :], ps[:, :], AF.Relu,
                                     scale=dv_t[:, vb:vb + 1])
                if li > 0:
                    hp = wrk.tile([P, H], F32, tag="hp")
                    nc.sync.dma_start(hp[:, :], h_d[vb * P:(vb + 1) * P, :])
                    nc.vector.tensor_add(hn[:, :], hn[:, :], hp[:, :])
                if li < L - 1:
                    nc.sync.dma_start(h_d[vb * P:(vb + 1) * P, :], hn[:, :])

                if stage < 6:
                    continue
                # LN stats + normalize (row-major), affine+relu after transpose
                st6 = stat.tile([P, 6], F32, tag="st6")
                nc.vector.bn_stats(st6[:, :], hn[:, :])
                mv = stat.tile([P, 2], F32, tag="mv")
                nc.vector.bn_aggr(mv[:, :], st6[:, :])
                rstd = stat.tile([P, 1], F32, tag="rstd")
                nc.scalar.activation(rstd[:, :], mv[:, 1:2], AF.Sqrt,
                                     bias=epsc[:, :], scale=1.0)
                rinv = stat.tile([P, 1], F32, tag="rinv")
                nc.vector.reciprocal(rinv[:, :], rstd[:, :])
                nm = stat.tile([P, 1], F32, tag="nm")
                nc.vector.tensor_scalar(nm[:, :], mv[:, 0:1], rinv[:, :], -1.0,
                                        op0=ALU.mult, op1=ALU.mult)
                zt = wrk.tile([P, H], BF16, tag="zt")
                nc.scalar.activation(zt[:, :], hn[:, :], AF.Identity,
                                     bias=nm[:, :], scale=rinv[:, :])
                for m in range(KH):
                    pst = psR.tile([P, P], BF16, tag="psR")
                    nc.tensor.transpose(pst[:, :], zt[:, m * P:(m + 1) * P], ident[:, :])
                    nc.scalar.activation(hT_b[vb][:, m, :], pst[:, :],
                                         AF.Relu, bias=lnB_c[lnxt][m][:, :],
                                         scale=lnG_c[lnxt][m][:, :])
                if li < L - 1:
                    emit_T_block(li + 1, vb)
                else:
                    emit_final_block(vb)

    nc.compile()
    return nc


# ----------------------------------------------------------------------------
# Full pipeline: prep + build + run
# ----------------------------------------------------------------------------
def run_full(x, vidx, eidx, encW, encB, thetaW, thetaB, lnG, lnB, linW, linB,
             N, E, C, trace=False, nc_cache=None, stage=99, **runkw):
    IN_DIM = x.shape[1]
    H = encW.shape[1]
    OUT = linW.shape[1]
    L = thetaW.shape[0]

    p = host_prep(np.asarray(vidx), np.asarray(eidx), N, E, C)
    nc = nc_cache if nc_cache is not None else build_program(p, IN_DIM, H, OUT, L, stage=stage)

    x = np.asarray(x, np.float32)
    NP, NPAD = p.NP, p.NPAD
    in_maps = []
    for c in range(C):
        xs = x[c * NP:(c + 1) * NP]
        xT = np.zeros((IN_DIM, NPAD), BF16_NP)
        xT[:, p.nodemap[c]] = xs.T.astype(BF16_NP)
        in_maps.append(dict(
            xT=xT,
            encW=np.asarray(encW, np.float32).astype(BF16_NP),
            encB=np.asarray(encB, np.float32),
            thW=np.asarray(thetaW, np.float32).astype(BF16_NP),
            thB=np.asarray(thetaB, np.float32).astype(BF16_NP),
            lnG=np.asarray(lnG, np.float32),
            lnB=np.asarray(lnB, np.float32),
            linW=np.asarray(linW, np.float32).astype(BF16_NP),
            linB=np.asarray(linB, np.float32).astype(BF16_NP),
            idxA=p.idxA_w[c],
            rA=p.rA_m[c],
            idxB=p.idxB_w[c],
            rB=p.rB_m[c],
            dvc=p.dvc[c],
            dec=p.dec,
        ))

    res = run_bass_kernel_spmd(nc, in_maps, core_ids=list(range(C)), trace=trace, **runkw)
    outs = []
    for c in range(C):
        op = res.results[c]["out"]  # [NPAD, OUT] in permuted order
        outs.append(op[p.nodemap[c]])
    out = np.concatenate(outs, axis=0)
    return out, res, nc, p


# hardcoded problem configuration (nn_DeeperHNN_88295937671288)
_N, _E, _NNZ = 100000, 20000, 800000
_C = 8

_nc_cache = None


def kernel(x, vidx, eidx, encW, encB, thetaW, thetaB, lnG, lnB, linW, linB):
    global _nc_cache
    out, res, nc, p = run_full(
        x, vidx, eidx, encW, encB, thetaW, thetaB, lnG, lnB, linW, linB,
        N=_N, E=_E, C=_C, nc_cache=None,
    )
    _nc_cache = nc
    return out.astype(np.float32)


# revision 16
# speedup vs baseline: 2.5597x; 1.0046x over previous
"""DeeperHNN hypergraph message passing kernel for 8 Trainium2 NeuronCores.

Strategy (sharding_hint): nodes (and incidence entries, partitioned by vertex)
are sharded across 8 cores; hyperedge aggregates are computed as per-core
partials and AllReduced (replicated) since E << N; weight matrices replicated.

v2 design notes (vs the first working version):
  * One dma_gather call per 128-block (<=1024 indices), rotated across the 4
    SWDGE queues, G-pool bufs=5 so several gathers are in flight.
  * Pad tokens use index -1 (skipped by the gather ucode: no descriptors) and
    one-hot row id -1 (builds an all-zero selector row), so garbage SBUF data
    from skipped descriptors never contributes.
  * Host-side bin packing: edges are renumbered globally and nodes renumbered
    per-core so per-(block, core) token counts are balanced -> fewer slots.
  * All PE matmuls in bf16 (fp32 is 4 cycles/row).  hT is SBUF-resident.
  * LN gain/bias/relu folded into the feature-major psum-read activation after
    the PE transpose (per-partition scale/bias).
  * T_{i+1} = h_{i+1} @ thetaW computed inside layer i's phase-B block loop;
    the final linear layer likewise inside layer 3's loop.
  * The Ye AllReduce is split into 4 edge-range chunks overlapping phase A.
"""

import numpy as np

import concourse.bacc as bacc
import concourse.bass as bass
import concourse.mybir as mybir
import concourse.tile as tile
from concourse.bass_utils import run_bass_kernel_spmd
from concourse.masks import make_identity

import ml_dtypes

P = 128
BF16_NP = ml_dtypes.bfloat16
F32 = mybir.dt.float32
BF16 = mybir.dt.bfloat16
I16 = mybir.dt.int16
I32 = mybir.dt.int32
AF = mybir.ActivationFunctionType
ALU = mybir.AluOpType


PAD_IDX = 0  # pad token row: -1 skips descriptors, >=0 gathers a live row


def _cdiv(a, b):
    return (a + b - 1) // b


# ----------------------------------------------------------------------------
# Host-side preprocessing: balanced block packing + token tables.
# ----------------------------------------------------------------------------
class Prep:
    pass


def _pack_first_fit(sizes, order, nblocks, item_cap, load_cap):
    """First-fit-decreasing of items into blocks with an item-count cap and a
    load cap per block; items that fit nowhere go to the min-load block.
    sizes: [n] (or [n, C] for multi-dim loads). Returns (block, pos) per item."""
    n = len(order)
    multi = sizes.ndim == 2
    C = sizes.shape[1] if multi else 1
    loads = np.zeros((nblocks, C), np.int64)
    nitem = np.zeros(nblocks, np.int64)
    blk = np.empty(n, np.int64)
    pos = np.empty(n, np.int64)
    for i in order:
        d = sizes[i] if multi else sizes[i : i + 1]
        ok = nitem < item_cap
        fits = ok & ((loads + d) <= load_cap).all(axis=1)
        if fits.any():
            b = int(np.argmax(fits))
        else:
            m = np.where(ok[:, None], loads + d, 1 << 60).max(axis=1)
            b = int(np.argmin(m))
        blk[i] = b
        pos[i] = nitem[b]
        nitem[b] += 1
        loads[b] += d
    return blk, pos


def host_prep(vidx, eidx, N, E, C):
    p = Prep()
    NP = N // C
    NBV = _cdiv(NP, P)
    NBE = _cdiv(E, P)
    NPAD = NBV * P
    EPAD = NBE * P
    p.N, p.E, p.C, p.NP, p.NBE, p.NBV, p.NPAD, p.EPAD = N, E, C, NP, NBE, NBV, NPAD, EPAD

    vidx = np.asarray(vidx).astype(np.int64)
    eidx = np.asarray(eidx).astype(np.int64)
    de = np.bincount(eidx, minlength=E)
    dv = np.bincount(vidx, minlength=N)
    de_inv = (1.0 / np.maximum(de, 1.0)).astype(np.float32)
    dv_inv = (1.0 / np.maximum(dv, 1.0)).astype(np.float32)
    core = vidx // NP

    # ---- global edge renumbering: balance per-core counts per edge block ----
    cnt_ce = np.bincount(core * E + eidx, minlength=C * E).reshape(C, E)
    eorder = np.argsort(-de, kind="stable")
    eblk, epos = _pack_first_fit(cnt_ce.T, eorder, NBE, P, 5 * P)
    enew = eblk * P + epos  # orig edge id -> new edge id
    dec = np.zeros(EPAD, np.float32)
    dec[enew] = de_inv
    p.dec = dec.reshape(NBE, P).T.copy()

    # ---- per-core node renumbering: balance token counts per node block ----
    nodemap = np.empty((C, NP), np.int64)  # orig local id -> new local id
    invnode = np.full((C, NPAD), -1, np.int64)  # new local id -> orig local id
    dvc = np.zeros((C, P, NBV), np.float32)
    for c in range(C):
        dl = dv[c * NP : (c + 1) * NP]
        norder = np.argsort(-dl, kind="stable")
        nblk, npos = _pack_first_fit(dl.astype(np.int64), norder, NBV, P, 8 * P)
        nn = nblk * P + npos
        nodemap[c] = nn
        invnode[c, nn] = np.arange(NP)
        col = np.zeros(NPAD, np.float32)
        col[nn] = dv_inv[c * NP : (c + 1) * NP]
        dvc[c] = col.reshape(NBV, P).T
    p.nodemap = nodemap
    p.invnode = invnode
    p.dvc = dvc

    # ---- phase A tokens: per core, sorted by new edge id ----
    cntA = np.zeros((C, NBE), np.int64)
    A_ev, A_lv = [], []
    for c in range(C):
        m = core == c
        ev = enew[eidx[m]]
        lv = nodemap[c, vidx[m] - c * NP]
        o = np.argsort(ev, kind="stable")
        ev, lv = ev[o], lv[o]
        cntA[c] = np.bincount(ev // P, minlength=NBE)
        A_ev.append(ev)
        A_lv.append(lv)
    slotsA = np.maximum(1, _cdiv(cntA.max(0), P)).astype(np.int64)
    SA = int(slotsA.sum())
    TA = SA * P
    offA = np.zeros(NBE + 1, np.int64)
    np.cumsum(slotsA * P, out=offA[1:])

    idxA = np.full((C, TA), PAD_IDX, np.int16)
    rA = np.full((C, TA), -1.0, np.float32)
    for c in range(C):
        ev, lv = A_ev[c], A_lv[c]
        blk = ev // P
        starts = np.searchsorted(ev, np.arange(NBE) * P)
        tok = offA[blk] + (np.arange(len(ev)) - starts[blk])
        idxA[c, tok] = lv
        rA[c, tok] = ev - blk * P

    # ---- phase B tokens: per core, sorted by new local node id ----
    cntB = np.zeros((C, NBV), np.int64)
    B_ee, B_lv = [], []
    for c in range(C):
        m = core == c
        lv = nodemap[c, vidx[m] - c * NP]
        ee = enew[eidx[m]]
        o = np.argsort(lv, kind="stable")
        lv, ee = lv[o], ee[o]
        cntB[c] = np.bincount(lv // P, minlength=NBV)
        B_ee.append(ee)
        B_lv.append(lv)
    slotsB = np.maximum(1, _cdiv(cntB.max(0), P)).astype(np.int64)
    SB = int(slotsB.sum())
    TB = SB * P
    offB = np.zeros(NBV + 1, np.int64)
    np.cumsum(slotsB * P, out=offB[1:])

    idxB = np.full((C, TB), PAD_IDX, np.int16)
    rB = np.full((C, TB), -1.0, np.float32)
    for c in range(C):
        ee, lv = B_ee[c], B_lv[c]
        blk = lv // P
        starts = np.searchsorted(lv, np.arange(NBV) * P)
        tok = offB[blk] + (np.arange(len(lv)) - starts[blk])
        idxB[c, tok] = ee
        rB[c, tok] = lv - blk * P

    p.slotsA, p.slotsB, p.SA, p.SB, p.TA, p.TB = slotsA, slotsB, SA, SB, TA, TB
    p.offA, p.offB = offA, offB
    p.idxA_w = np.ascontiguousarray(
        np.tile(idxA.reshape(C, TA // 16, 16).transpose(0, 2, 1), (1, 8, 1))
    )
    p.rA_m = np.ascontiguousarray(rA.reshape(C, SA, P).transpose(0, 2, 1)).astype(BF16_NP)
    p.idxB_w = np.ascontiguousarray(
        np.tile(idxB.reshape(C, TB // 16, 16).transpose(0, 2, 1), (1, 8, 1))
    )
    p.rB_m = np.ascontiguousarray(rB.reshape(C, SB, P).transpose(0, 2, 1)).astype(BF16_NP)
    p.MAXSLOT = int(max(slotsA.max(), slotsB.max()))
    return p


# ----------------------------------------------------------------------------
# Device program
# ----------------------------------------------------------------------------
def build_program(p, IN_DIM, H, OUT, L, stage=99):
    C, NBE, NBV, NPAD, EPAD = p.C, p.NBE, p.NBV, p.NPAD, p.EPAD
    KI = IN_DIM // P
    KH = H // P
    assert IN_DIM % P == 0 and H % P == 0
    MS = p.MAXSLOT
    pairA = [int(p.slotsA[i] + (p.slotsA[i + 1] if i + 1 < NBE else 0))
             for i in range(0, NBE, 2)]
    GMS = max(max(pairA), int(p.slotsB.max()))

    nc = bacc.Bacc(
        "TRN2",
        target_bir_lowering=False,
        debug=False,
        enable_asserts=False,
        num_devices=C,
        num_swdge_queues=4,
    )

    # ---- I/O ----
    xT_d = nc.dram_tensor("xT", [IN_DIM, NPAD], BF16, kind="ExternalInput")
    encW_d = nc.dram_tensor("encW", [IN_DIM, H], BF16, kind="ExternalInput")
    encB_d = nc.dram_tensor("encB", [H], F32, kind="ExternalInput")
    thW_d = nc.dram_tensor("thW", [L, H, H], BF16, kind="ExternalInput")
    thB_d = nc.dram_tensor("thB", [L, H], BF16, kind="ExternalInput")
    lnG_d = nc.dram_tensor("lnG", [L, H], F32, kind="ExternalInput")
    lnB_d = nc.dram_tensor("lnB", [L, H], F32, kind="ExternalInput")
    linW_d = nc.dram_tensor("linW", [H, OUT], BF16, kind="ExternalInput")
    linB_d = nc.dram_tensor("linB", [OUT], BF16, kind="ExternalInput")
    idxA_d = nc.dram_tensor("idxA", [P, p.TA // 16], I16, kind="ExternalInput")
    rA_d = nc.dram_tensor("rA", [P, p.SA], BF16, kind="ExternalInput")
    idxB_d = nc.dram_tensor("idxB", [P, p.TB // 16], I16, kind="ExternalInput")
    rB_d = nc.dram_tensor("rB", [P, p.SB], BF16, kind="ExternalInput")
    dv_d = nc.dram_tensor("dvc", [P, NBV], F32, kind="ExternalInput")
    dec_d = nc.dram_tensor("dec", [P, NBE], F32, kind="ExternalInput")
    out_d = nc.dram_tensor("out", [NPAD, OUT], F32, kind="ExternalOutput")

    # ---- internals ----
    T_d = nc.dram_tensor("T_t", [NPAD, H], BF16)
    YeP_d = nc.dram_tensor("YeP", [EPAD, H], BF16)
    YeF_d = nc.dram_tensor("YeF", [EPAD, H], BF16, addr_space="Shared")

    # AllReduce chunk boundaries in edge-block units
    AR_BLKS = [0, 40, 80, 120, NBE]

    from contextlib import ExitStack
    with tile.TileContext(nc) as tc, ExitStack() as es:
        const = es.enter_context(tc.tile_pool(name="const", bufs=1))
        meta = es.enter_context(tc.tile_pool(name="meta", bufs=1))
        hTp = es.enter_context(tc.tile_pool(name="hTp", bufs=1))
        gpool = es.enter_context(tc.tile_pool(name="gpool", bufs=5))
        spool = es.enter_context(tc.tile_pool(name="spool", bufs=4))
        wrk = es.enter_context(tc.tile_pool(name="wrk", bufs=3))
        stat = es.enter_context(tc.tile_pool(name="stat", bufs=4))
        opool = es.enter_context(tc.tile_pool(name="opool", bufs=3))
        psA = es.enter_context(tc.tile_pool(name="psA", bufs=3, space="PSUM"))
        psT = es.enter_context(tc.tile_pool(name="psT", bufs=2, space="PSUM"))
        psR = es.enter_context(tc.tile_pool(name="psR", bufs=2, space="PSUM"))
        psE = es.enter_context(tc.tile_pool(name="psE", bufs=1, space="PSUM"))

        # ---- constants ----
        iota_i = const.tile([P, P], I32)
        nc.gpsimd.iota(iota_i[:, :], pattern=[[1, P]], base=0,
                       channel_multiplier=0)
        iota_f = const.tile([P, P], BF16)
        nc.vector.tensor_copy(iota_f[:, :], iota_i[:, :])
        ident = const.tile([P, P], BF16)
        make_identity(nc, ident[:, :])
        ones1 = const.tile([1, P], BF16)
        nc.vector.memset(ones1[:, :], 1.0)
        epsc = const.tile([P, 1], F32)
        nc.vector.memset(epsc[:, :], 1e-5)

        # weights (bf16 on-chip)
        encW_t = []
        for k in range(KI):
            row = []
            for m in range(KH):
                t = const.tile([P, P], BF16, tag=f"encW{k}{m}")
                nc.sync.dma_start(t[:, :], encW_d[k * P:(k + 1) * P, m * P:(m + 1) * P])
                row.append(t)
            encW_t.append(row)
        encB_c = []
        for m in range(KH):
            t = const.tile([P, 1], F32, tag=f"encB{m}")
            nc.sync.dma_start(t[:, :], encB_d[m * P:(m + 1) * P, None])
            encB_c.append(t)
        thW_t = []
        for i in range(L):
            row = []
            for k in range(KH):
                t = const.tile([P, H], BF16, tag=f"thW{i}{k}")
                nc.sync.dma_start(t[:, :], thW_d[i, k * P:(k + 1) * P, :])
                row.append(t)
            thW_t.append(row)
        thB_t = []
        for i in range(L):
            t = const.tile([1, H], BF16, tag=f"thB{i}")
            nc.sync.dma_start(t[:, :], thB_d[i:i + 1, :])
            thB_t.append(t)
        linW_t = []
        for k in range(KH):
            t = const.tile([P, OUT], BF16, tag=f"linW{k}")
            nc.sync.dma_start(t[:, :], linW_d[k * P:(k + 1) * P, :])
            linW_t.append(t)
        linB_t = const.tile([1, OUT], BF16)
        nc.sync.dma_start(linB_t[:, :], linB_d[None, :])
        # LN gain/bias as feature-major per-partition columns
        lnG_c, lnB_c = [], []
        for i in range(L):
            gs, bs = [], []
            for m in range(KH):
                g = const.tile([P, 1], F32, tag=f"lnG{i}{m}")
                b = const.tile([P, 1], F32, tag=f"lnB{i}{m}")
                nc.sync.dma_start(g[:, :], lnG_d[i, m * P:(m + 1) * P, None])
                nc.sync.dma_start(b[:, :], lnB_d[i, m * P:(m + 1) * P, None])
                gs.append(g)
                bs.append(b)
            lnG_c.append(gs)
            lnB_c.append(bs)

        # metadata
        idxA_t = meta.tile([P, p.TA // 16], I16)
        nc.sync.dma_start(idxA_t[:, :], idxA_d[:, :])
        rA_t = meta.tile([P, p.SA], BF16)
        nc.sync.dma_start(rA_t[:, :], rA_d[:, :])
        dec_t = meta.tile([P, NBE], F32)
        nc.sync.dma_start(dec_t[:, :], dec_d[:, :])
        idxB_t = meta.tile([P, p.TB // 16], I16)
        nc.sync.dma_start(idxB_t[:, :], idxB_d[:, :])
        rB_t = meta.tile([P, p.SB], BF16)
        nc.sync.dma_start(rB_t[:, :], rB_d[:, :])
        dv_t = meta.tile([P, NBV], F32)
        nc.sync.dma_start(dv_t[:, :], dv_d[:, :])

        # resident transposed activations hT [feat, nodes] (bf16), one tile
        # per node block so each region has a single writer
        hT_b = [hTp.tile([P, KH, P], BF16, tag=f"hT{vb}", name=f"hT{vb}")
                for vb in range(NBV)]
        h_b = [hTp.tile([P, H], BF16, tag=f"h{vb}", name=f"h{vb}")
               for vb in range(NBV)]

        qn = [0]

        def next_q():
            q = qn[0]
            qn[0] = (q + 1) % 4
            return q

        def emit_gather(G, src_d, idx_t, s0, sb):
            g0 = 0
            while g0 < sb:
                gs = min(8, sb - g0)
                tok0 = (s0 + g0) * P
                nc.gpsimd.dma_gather(
                    out_ap=G[:, g0:g0 + gs, :],
                    in_ap=src_d[:, :],
                    idxs_ap=idx_t[:, tok0 // 16:(tok0 + gs * P) // 16],
                    num_idxs=gs * P,
                    num_idxs_reg=gs * P,
                    elem_size=H,
                    queue_num=next_q(),
                )
                g0 += gs

        def emit_T_block(li, vb):
            """T_d[vb block] = t @ thetaW[li] + thetaB[li], t from resident hT."""
            ps = psT.tile([P, H], F32, tag="psT")
            for k in range(KH):
                nc.tensor.matmul(ps[:, :], lhsT=hT_b[vb][:, k, :],
                                 rhs=thW_t[li][k][:, :],
                                 start=(k == 0), stop=False)
            nc.tensor.matmul(ps[:, :], lhsT=ones1[:1, :], rhs=thB_t[li][:1, :],
                             start=False, stop=True)
            Tb = opool.tile([P, H], BF16, tag="Tout")
            nc.scalar.activation(Tb[:, :], ps[:, :], AF.Copy)
            nc.sync.dma_start(T_d[vb * P:(vb + 1) * P, :], Tb[:, :])

        def emit_final_block(vb):
            """out_d[vb block] = t @ linW + linB, t from resident hT."""
            ps = psT.tile([P, H], F32, tag="psT")
            for k in range(KH):
                nc.tensor.matmul(ps[:, :OUT], lhsT=hT_b[vb][:, k, :],
                                 rhs=linW_t[k][:, :],
                                 start=(k == 0), stop=False)
            nc.tensor.matmul(ps[:, :OUT], lhsT=ones1[:1, :], rhs=linB_t[:1, :],
                             start=False, stop=True)
            ob = opool.tile([P, OUT], F32, tag="finout")
            nc.scalar.activation(ob[:, :], ps[:, :OUT], AF.Copy)
            nc.sync.dma_start(out_d[vb * P:(vb + 1) * P, :], ob[:, :])

        # ------------------------------------------------------------------
        # Encoder: hT = (x @ encW + encB)^T directly feature-major, and
        # layer-0 T blocks as soon as their hT columns exist.
        # ------------------------------------------------------------------
        CW = 512
        for c0 in range(0, NPAD, CW):
            ncols = min(CW, NPAD - c0)
            xc = wrk.tile([P, KI, CW], BF16, tag="xc")
            nc.sync.dma_start(
                xc[:, :, :ncols],
                xT_d.ap().rearrange("(k q) n -> q k n", q=P)[:, :, c0:c0 + ncols],
            )
            for m in range(KH):
                ps = psE.tile([P, CW], F32, tag="psE")
                for k in range(KI):
                    nc.tensor.matmul(ps[:, :ncols], lhsT=encW_t[k][m][:, :],
                                     rhs=xc[:, k, :ncols],
                                     start=(k == 0), stop=(k == KI - 1))
                for j in range(ncols // P):
                    nc.scalar.activation(hT_b[c0 // P + j][:, m, :],
                                         ps[:, j * P:(j + 1) * P],
                                         AF.Identity, bias=encB_c[m][:, :], scale=1.0)
            if stage >= 2:
                for vb in range(c0 // P, (c0 + ncols) // P):
                    emit_T_block(0, vb)

        # ------------------------------------------------------------------
        # Conv layers
        # ------------------------------------------------------------------
        for li in range(L if stage >= 3 else 0):
            # ---- Phase A: partial Ye, one gather call per PAIR of blocks ----
            ar_next = 1
            for eb0 in range(0, NBE, 2):
                npair = min(2, NBE - eb0)
                sbs = [int(p.slotsA[eb0 + j]) for j in range(npair)]
                s0 = int(p.offA[eb0]) // P
                sbt = sum(sbs)
                G = gpool.tile([P, GMS, H], BF16, tag="G")
                emit_gather(G, T_d, idxA_t, s0, sbt)
                S = spool.tile([P, GMS, P], BF16, tag="S")
                iota_ap = iota_f[:, :].unsqueeze(1).broadcast_to([P, sbt, P])
                rb_ap = rA_t[:, s0:s0 + sbt].unsqueeze(2).broadcast_to([P, sbt, P])
                nc.vector.tensor_tensor(S[:, :sbt, :], iota_ap, rb_ap,
                                        op=ALU.is_equal)
                so = 0
                for j in range(npair):
                    eb = eb0 + j
                    sb = sbs[j]
                    ps = psA.tile([P, H], F32, tag="psA")
                    for s in range(sb):
                        nc.tensor.matmul(ps[:, :], lhsT=S[:, so + s, :],
                                         rhs=G[:, so + s, :],
                                         start=(s == 0), stop=(s == sb - 1))
                    so += sb
                    yeb = opool.tile([P, H], BF16, tag="yeg")
                    nc.scalar.activation(yeb[:, :], ps[:, :], AF.Copy,
                                         scale=dec_t[:, eb:eb + 1])
                    nc.scalar.dma_start(YeP_d[eb * P:(eb + 1) * P, :], yeb[:, :])
                eb = eb0 + npair - 1
                if stage >= 4 and ar_next < len(AR_BLKS) and eb == AR_BLKS[ar_next] - 1:
                    r0, r1 = AR_BLKS[ar_next - 1] * P, AR_BLKS[ar_next] * P
                    nc.gpsimd.collective_compute(
                        "AllReduce",
                        ALU.add,
                        replica_groups=[list(range(C))],
                        ins=[YeP_d.ap()[r0:r1, :]],
                        outs=[YeF_d.ap()[r0:r1, :]],
                    )
                    ar_next += 1

            if stage < 5:
                continue

            # ---- Phase B: conv + residual + LN tail + next-layer T ----
            lnxt = li + 1 if li + 1 < L else 0
            for vb in range(NBV):
                sb = int(p.slotsB[vb])
                s0 = int(p.offB[vb]) // P
                G = gpool.tile([P, GMS, H], BF16, tag="G")
                emit_gather(G, YeF_d, idxB_t, s0, sb)
                S = spool.tile([P, GMS, P], BF16, tag="S")
                iota_ap = iota_f[:, :].unsqueeze(1).broadcast_to([P, sb, P])
                rb_ap = rB_t[:, s0:s0 + sb].unsqueeze(2).broadcast_to([P, sb, P])
                nc.vector.tensor_tensor(S[:, :sb, :], iota_ap, rb_ap,
                                        op=ALU.is_equal)
                ps = psA.tile([P, H], F32, tag="psA")
                for s in range(sb):
                    nc.tensor.matmul(ps[:, :], lhsT=S[:, s, :], rhs=G[:, s, :],
                                     start=(s == 0), stop=(s == sb - 1))
                # relu(dv * x) == dv * relu(x) since dv >= 0
                if li == 0:
                    nc.scalar.activation(h_b[vb][:, :], ps[:, :], AF.Relu,
                                         scale=dv_t[:, vb:vb + 1])
                else:
                    cnv = wrk.tile([P, H], BF16, tag="cnv")
                    nc.scalar.activation(cnv[:, :], ps[:, :], AF.Relu,
                                         scale=dv_t[:, vb:vb + 1])
                    nc.vector.tensor_tensor(h_b[vb][:, :], cnv[:, :], h_b[vb][:, :],
                                            op=ALU.add)

                if stage < 6:
                    continue
                # LN stats + normalize (row-major), affine+relu after transpose
                st6 = stat.tile([P, 6], F32, tag="st6")
                nc.vector.bn_stats(st6[:, :], h_b[vb][:, :])
                mv = stat.tile([P, 2], F32, tag="mv")
                nc.vector.bn_aggr(mv[:, :], st6[:, :])
                rstd = stat.tile([P, 1], F32, tag="rstd")
                nc.scalar.activation(rstd[:, :], mv[:, 1:2], AF.Sqrt,
                                     bias=epsc[:, :], scale=1.0)
                rinv = stat.tile([P, 1], F32, tag="rinv")
                nc.vector.reciprocal(rinv[:, :], rstd[:, :])
                nm = stat.tile([P, 1], F32, tag="nm")
                nc.vector.tensor_scalar(nm[:, :], mv[:, 0:1], rinv[:, :], -1.0,
                                        op0=ALU.mult, op1=ALU.mult)
                zt = wrk.tile([P, H], BF16, tag="zt")
                nc.scalar.activation(zt[:, :], h_b[vb][:, :], AF.Identity,
                                     bias=nm[:, :], scale=rinv[:, :])
                for m in range(KH):
                    pst = psR.tile([P, P], BF16, tag="psR")
                    nc.tensor.transpose(pst[:, :], zt[:, m * P:(m + 1) * P], ident[:, :])
                    nc.scalar.activation(hT_b[vb][:, m, :], pst[:, :],
                                         AF.Relu, bias=lnB_c[lnxt][m][:, :],
                                         scale=lnG_c[lnxt][m][:, :])
                if li < L - 1:
                    emit_T_block(li + 1, vb)
                else:
                    emit_final_block(vb)

    nc.compile()
    return nc


# ----------------------------------------------------------------------------
# Full pipeline: prep + build + run
# ----------------------------------------------------------------------------
def run_full(x, vidx, eidx, encW, encB, thetaW, thetaB, lnG, lnB, linW, linB,
             N, E, C, trace=False, nc_cache=None, stage=99, **runkw):
    IN_DIM = x.shape[1]
    H = encW.shape[1]
    OUT = linW.shape[1]
    L = thetaW.shape[0]

    p = host_prep(np.asarray(vidx), np.asarray(eidx), N, E, C)
    nc = nc_cache if nc_cache is not None else build_program(p, IN_DIM, H, OUT, L, stage=stage)

    x = np.asarray(x, np.float32)
    NP, NPAD = p.NP, p.NPAD
    in_maps = []
    for c in range(C):
        xs = x[c * NP:(c + 1) * NP]
        xT = np.zeros((IN_DIM, NPAD), BF16_NP)
        xT[:, p.nodemap[c]] = xs.T.astype(BF16_NP)
        in_maps.append(dict(
            xT=xT,
            encW=np.asarray(encW, np.float32).astype(BF16_NP),
            encB=np.asarray(encB, np.float32),
            thW=np.asarray(thetaW, np.float32).astype(BF16_NP),
            thB=np.asarray(thetaB, np.float32).astype(BF16_NP),
            lnG=np.asarray(lnG, np.float32),
            lnB=np.asarray(lnB, np.float32),
            linW=np.asarray(linW, np.float32).astype(BF16_NP),
            linB=np.asarray(linB, np.float32).astype(BF16_NP),
            idxA=p.idxA_w[c],
            rA=p.rA_m[c],
            idxB=p.idxB_w[c],
            rB=p.rB_m[c],
            dvc=p.dvc[c],
            dec=p.dec,
        ))

    res = run_bass_kernel_spmd(nc, in_maps, core_ids=list(range(C)), trace=trace, **runkw)
    outs = []
    for c in range(C):
        op = res.results[c]["out"]  # [NPAD, OUT] in permuted order
        outs.append(op[p.nodemap[c]])
    out = np.concatenate(outs, axis=0)
    return out, res, nc, p


# hardcoded problem configuration (nn_DeeperHNN_88295937671288)
_N, _E, _NNZ = 100000, 20000, 800000
_C = 8

_nc_cache = None


def kernel(x, vidx, eidx, encW, encB, thetaW, thetaB, lnG, lnB, linW, linB):
    global _nc_cache
    out, res, nc, p = run_full(
        x, vidx, eidx, encW, encB, thetaW, thetaB, lnG, lnB, linW, linB,
        N=_N, E=_E, C=_C, nc_cache=None,
    )
    _nc_cache = nc
    return out.astype(np.float32)
